# revision 16
# baseline (speedup 1.0000x reference)
"""Trainium2 Bass kernel for EquivariantGraphConv message passing.

Math (reference):
    scalar = x[:,0,:]; vector = x[:,1:,:].reshape(N, 3H)
    scalar_out = scalar @ Wsr.T + b + segsum(scalar[col] @ Wsrel.T, row)
    vector_out = vector @ Wvr.T + segsum(vector[col] @ Wvrel.T, row)

Key identity used: the edge transform is linear, so
    segsum(feat[col] @ W.T, row) == segsum(feat[col], row) @ W.T
We therefore aggregate the raw 512-dim node features per destination first
(16x fewer matmul FLOPs), then apply all four weight matrices per *node*.

Sharding: destinations are sharded across the 8 cores (1280 nodes each, in
10 chunks of 128).  Edges are sorted by destination on the host, so each
core only consumes its own edge shard and no cross-core reduction is
needed.  Each core gathers source features from a replicated padded node
table in DRAM with one big indirect DMA per chunk, builds one-hot
"selection" matrices on the vector engine (row_in_chunk == iota) and
matmul-accumulates P^T @ G into PSUM to realize the segment sum.
"""

import os
import sys

sys.path.insert(0, "/opt/trn_rl_repo")

import numpy as np
import ml_dtypes

import concourse.bass as bass
import concourse.mybir as mybir
import concourse.tile as tile
from concourse.bacc import Bacc
from concourse.bass_utils import run_bass_kernel_spmd

N_NODES = 10000
N_EDGES = 160000
H = 128
F = 4 * H            # 512 features per node (scalar 128 + vector 384)
P = 128              # partitions
NP_PAD = 10240       # padded node count (80 chunks of 128)
N_CORES = 8
NODES_PER_CORE = NP_PAD // N_CORES       # 1280
CHUNKS_PER_CORE = NODES_PER_CORE // P    # 10
N_CHUNKS = NP_PAD // P                   # 80
ZERO_ROW = N_NODES                       # padded zero row used by dummy edges
DEFAULT_T = 18                           # edge tiles per chunk (18*128 = 2304 cap)

# configuration: (gather/stage1 dtype, stage2 dtype); each of
# "bf16" | "f32" | "f32r".  f32r = fp32 storage, TF32-like matmul.
CFG = os.environ.get("BASS_GNN_CFG", "bf16,bf16")

# test.py hooks
PROFILE = {"on": False, "trace_cores": None, "last": None}

_prog_cache = {}


def _dt(name):
    return {
        "bf16": mybir.dt.bfloat16,
        "f32": mybir.dt.float32,
        "f32r": mybir.dt.float32,  # storage dtype; bitcast at matmul time
    }[name]


def _npdt(name):
    return {
        "bf16": ml_dtypes.bfloat16,
        "f32": np.float32,
        "f32r": np.float32,
    }[name]


def _build_program(T, cfg):
    """Build the (SPMD, per-core-identical) Bass program."""
    s1_name, s2_name = cfg
    s1_store = _dt(s1_name)
    s2_store = _dt(s2_name)

    def s1(ap):
        return ap.bitcast(mybir.dt.float32r) if s1_name == "f32r" else ap

    def s2(ap):
        return ap.bitcast(mybir.dt.float32r) if s2_name == "f32r" else ap

    nc = Bacc("TRN2")
    f32 = mybir.dt.float32

    xg = nc.dram_tensor("xg", [NP_PAD, F], s1_store, kind="ExternalInput")
    # dma_gather is limited to ~1024 descriptors per instruction; split each
    # chunk's T*128 indices into NQ pieces of <= GQ indices.
    GQ = 1024
    NQ = (T * P + GQ - 1) // GQ
    WQ = GQ // 16  # idx columns per piece in the 16-partition wrapped layout
    cols = nc.dram_tensor("cols", [CHUNKS_PER_CORE, P, NQ * WQ], mybir.dt.int16,
                          kind="ExternalInput")
    rr = nc.dram_tensor("rr", [CHUNKS_PER_CORE, P, T], f32,
                        kind="ExternalInput")
    xt = nc.dram_tensor("xt", [P, 4 * NODES_PER_CORE], s2_store,
                        kind="ExternalInput")
    wsrel = nc.dram_tensor("wsrel", [P, H], s2_store, kind="ExternalInput")
    wsroot = nc.dram_tensor("wsroot", [P, H], s2_store, kind="ExternalInput")
    wvrel = nc.dram_tensor("wvrel", [P, 3 * 384], s2_store, kind="ExternalInput")
    wvroot = nc.dram_tensor("wvroot", [P, 3 * 384], s2_store, kind="ExternalInput")
    bias = nc.dram_tensor("bias", [P, H], f32, kind="ExternalInput")
    iota = nc.dram_tensor("iota", [P, P], f32, kind="ExternalInput")
    ident = nc.dram_tensor("ident", [P, P], s2_store, kind="ExternalInput")
    out = nc.dram_tensor("out", [NODES_PER_CORE, F], f32, kind="ExternalOutput")
    debug = os.environ.get("BASS_GNN_DEBUG", "0") == "1"
    if debug:
        dbg_agg = nc.dram_tensor("dbg_agg", [NODES_PER_CORE, F], f32,
                                 kind="ExternalOutput")
        dbg_g = nc.dram_tensor("dbg_g", [P, T * F], f32, kind="ExternalOutput")
        dbg_p = nc.dram_tensor("dbg_p", [P, T * P], f32, kind="ExternalOutput")

    with tile.TileContext(nc) as tc:
        with (
            tc.tile_pool(name="consts", bufs=1) as cpool,
            tc.tile_pool(name="edges", bufs=2) as epool,
            tc.tile_pool(name="work", bufs=2) as wpool,
            tc.tile_pool(name="pagg", bufs=2, space="PSUM") as pagg,
            tc.tile_pool(name="pmisc", bufs=2, space="PSUM") as pmisc,
        ):
            xt_sb = cpool.tile([P, 4 * NODES_PER_CORE], s2_store)
            nc.sync.dma_start(xt_sb[:], xt[:])
            wsrel_sb = cpool.tile([P, H], s2_store)
            nc.sync.dma_start(wsrel_sb[:], wsrel[:])
            wsroot_sb = cpool.tile([P, H], s2_store)
            nc.sync.dma_start(wsroot_sb[:], wsroot[:])
            wvrel_sb = cpool.tile([P, 3 * 384], s2_store)
            nc.sync.dma_start(wvrel_sb[:], wvrel[:])
            wvroot_sb = cpool.tile([P, 3 * 384], s2_store)
            nc.sync.dma_start(wvroot_sb[:], wvroot[:])
            bias_sb = cpool.tile([P, H], f32)
            nc.sync.dma_start(bias_sb[:], bias[:])
            iota_sb = cpool.tile([P, P], f32)
            nc.sync.dma_start(iota_sb[:], iota[:])
            ident_sb = cpool.tile([P, P], s2_store)
            nc.sync.dma_start(ident_sb[:], ident[:])

            for c in range(CHUNKS_PER_CORE):
                cols_sb = epool.tile([P, NQ * WQ], mybir.dt.int16, tag="cols")
                nc.sync.dma_start(cols_sb[:], cols[c])
                rr_sb = epool.tile([P, T], f32, tag="rr")
                nc.sync.dma_start(rr_sb[:], rr[c])

                # gather: edge i -> G[i % 128, i // 128, :] = xg[cols_flat[i], :]
                G = epool.tile([P, T * F], s1_store, tag="G")
                for q in range(NQ):
                    nidx = min(GQ, T * P - q * GQ)
                    nslots = nidx // P
                    nc.gpsimd.dma_gather(
                        G[:, q * (GQ // P) * F:
                             (q * (GQ // P) + nslots) * F]
                        .rearrange("p (t f) -> p t f", f=F),
                        xg[:],
                        cols_sb[:, q * WQ:(q + 1) * WQ],
                        nidx,
                        nidx,
                        F,
                    )

                # one-hot P[p, t*128 + d] = (rr[p, t] == d)
                Pm = epool.tile([P, T * P], s1_store, tag="P")
                for t in range(T):
                    nc.vector.tensor_tensor(
                        out=Pm[:, t * P:(t + 1) * P],
                        in0=rr_sb[:, t:t + 1].to_broadcast([P, P]),
                        in1=iota_sb[:],
                        op=mybir.AluOpType.is_equal,
                    )

                # segment-sum: agg[d, f] = sum_t P_t^T @ G_t
                agg_ps = pagg.tile([P, F], f32, tag="agg")
                for t in range(T):
                    nc.tensor.matmul(
                        out=agg_ps[:],
                        lhsT=s1(Pm[:, t * P:(t + 1) * P]),
                        rhs=s1(G[:, t * F:(t + 1) * F]),
                        start=(t == 0),
                        stop=(t == T - 1),
                    )
                agg_sb = wpool.tile([P, F], s2_store, tag="aggsb")
                nc.vector.tensor_copy(agg_sb[:], agg_ps[:])
                if debug:
                    agg_f32_sb = wpool.tile([P, F], f32, tag="dbgagg")
                    nc.vector.tensor_copy(agg_f32_sb[:], agg_ps[:])
                    nc.sync.dma_start(dbg_agg[c * P:(c + 1) * P, :], agg_f32_sb[:])
                    if c == 0 and s1_store == f32:
                        nc.sync.dma_start(dbg_g[:], G[:])
                        nc.sync.dma_start(dbg_p[:], Pm[:])
                    elif c == 0:
                        g_f32_sb = wpool.tile([P, T * F], f32, tag="dbgg")
                        nc.vector.tensor_copy(g_f32_sb[:], G[:])
                        nc.sync.dma_start(dbg_g[:], g_f32_sb[:])
                        p_f32_sb = wpool.tile([P, T * P], f32, tag="dbgp")
                        nc.vector.tensor_copy(p_f32_sb[:], Pm[:])
                        nc.sync.dma_start(dbg_p[:], p_f32_sb[:])

                # transpose agg -> aggT[f, d] (4 PE transposes of 128x128)
                aggT_ps = pmisc.tile([P, F], s2_store, tag="aggT")
                for fc in range(4):
                    nc.tensor.transpose(
                        out=s2(aggT_ps[:, fc * P:(fc + 1) * P]),
                        in_=s2(agg_sb[:, fc * P:(fc + 1) * P]),
                        identity=s2(ident_sb[:]),
                    )
                aggT_sb = wpool.tile([P, F], s2_store, tag="aggTsb")
                nc.vector.tensor_copy(aggT_sb[:], aggT_ps[:])

                # stage 2: out[d, :128]  = agg_s @ WsrelT + x_s @ WsrootT (+bias)
                #          out[d, 128:]  = agg_v @ WvrelT + x_v @ WvrootT
                osv_ps = pmisc.tile([P, F], f32, tag="osv")
                nc.tensor.matmul(out=osv_ps[:, 0:H],
                                 lhsT=s2(aggT_sb[:, 0:P]), rhs=s2(wsrel_sb[:]),
                                 start=True, stop=False)
                nc.tensor.matmul(out=osv_ps[:, 0:H],
                                 lhsT=s2(xt_sb[:, c * P:(c + 1) * P]),
                                 rhs=s2(wsroot_sb[:]),
                                 start=False, stop=True)
                for kc in range(3):
                    nc.tensor.matmul(
                        out=osv_ps[:, H:F],
                        lhsT=s2(aggT_sb[:, (1 + kc) * P:(2 + kc) * P]),
                        rhs=s2(wvrel_sb[:, kc * 384:(kc + 1) * 384]),
                        start=(kc == 0), stop=False)
                for kc in range(3):
                    nc.tensor.matmul(
                        out=osv_ps[:, H:F],
                        lhsT=s2(xt_sb[:, (1 + kc) * NODES_PER_CORE + c * P:
                                      (1 + kc) * NODES_PER_CORE + (c + 1) * P]),
                        rhs=s2(wvroot_sb[:, kc * 384:(kc + 1) * 384]),
                        start=False, stop=(kc == 2))

                out_sb = wpool.tile([P, F], f32, tag="outsb")
                nc.vector.tensor_add(out_sb[:, 0:H], osv_ps[:, 0:H], bias_sb[:])
                nc.vector.tensor_copy(out_sb[:, H:F], osv_ps[:, H:F])
                nc.sync.dma_start(out[c * P:(c + 1) * P, :], out_sb[:])

    nc.finalize()
    return nc


def _get_program(T, cfg):
    key = (T, cfg)
    if key not in _prog_cache:
        _prog_cache[key] = _build_program(T, cfg)
    return _prog_cache[key]


def kernel(x, edge_index, W_scalar_rel, W_scalar_root, b_scalar_root,
           W_vector_rel, W_vector_root):
    cfg = tuple(CFG.split(","))
    s1_np = _npdt(cfg[0])
    s2_np = _npdt(cfg[1])

    x = np.asarray(x, dtype=np.float32)
    n = x.shape[0]
    assert n == N_NODES, x.shape
    row = np.asarray(edge_index[0], dtype=np.int64)
    col = np.asarray(edge_index[1], dtype=np.int64)

    # ---- host-side shard construction (sort edges by destination) ----
    order = np.argsort(row, kind="stable")
    row_s = row[order]
    col_s = col[order]
    bounds = np.searchsorted(row_s, np.arange(0, NP_PAD + 1, P))
    counts = np.diff(bounds)
    T = max(DEFAULT_T, int(np.ceil(counts.max() / P)))

    cap = T * P
    cols_pad = np.full((N_CHUNKS, cap), ZERO_ROW, dtype=np.int16)
    rr_pad = np.zeros((N_CHUNKS, cap), dtype=np.float32)
    for g in range(N_CHUNKS):
        s, e = bounds[g], bounds[g + 1]
        m = e - s
        if m:
            cols_pad[g, :m] = col_s[s:e]
            rr_pad[g, :m] = (row_s[s:e] - g * P).astype(np.float32)
    # dma_gather: flat edge i -> partition i % 128, tile-slot i // 128.
    # Each chunk's indices are split into NQ pieces of <= 1024; within a
    # piece, idx element j lives at wrapped position [j % 16, j // 16],
    # and the 16-row block is replicated across all 128 partitions
    # (the tx/rx Q7 cores each read their own 16-partition copy).
    GQ = 1024
    NQ = (cap + GQ - 1) // GQ
    WQ = GQ // 16
    cols_q = np.zeros((N_CHUNKS, NQ, GQ), dtype=np.int16)
    cols_q.reshape(N_CHUNKS, NQ * GQ)[:, :cap] = cols_pad
    wrap = cols_q.reshape(N_CHUNKS, NQ, WQ, 16).transpose(0, 1, 3, 2)  # [.., 16, WQ]
    cols_wrapped = np.tile(wrap, (1, 1, 8, 1))  # [N_CHUNKS, NQ, 128, WQ]
    cols_flat = cols_wrapped.transpose(0, 2, 1, 3).reshape(N_CHUNKS, P, NQ * WQ)
    cols_arr = np.ascontiguousarray(cols_flat).reshape(
        N_CORES, CHUNKS_PER_CORE, P, NQ * WQ)
    # rr for edge i goes to [i % 128, i // 128]
    rr_arr = rr_pad.reshape(N_CHUNKS, T, P).transpose(0, 2, 1)
    rr_arr = np.ascontiguousarray(rr_arr).reshape(N_CORES, CHUNKS_PER_CORE, P, T)

    x_flat = np.zeros((NP_PAD, F), dtype=np.float32)
    x_flat[:n] = x.reshape(n, F)
    xg_full = np.ascontiguousarray(x_flat.astype(s1_np))

    xT = x_flat.T  # [512, 10240], exact f32 for the root transform

    wsrelT = np.ascontiguousarray(np.asarray(W_scalar_rel, np.float32).T).astype(s2_np)
    wsrootT = np.ascontiguousarray(np.asarray(W_scalar_root, np.float32).T).astype(s2_np)
    wvrelT = np.ascontiguousarray(np.asarray(W_vector_rel, np.float32).T)
    wvrootT = np.ascontiguousarray(np.asarray(W_vector_root, np.float32).T)
    wvrel_packed = np.concatenate(
        [wvrelT[kc * P:(kc + 1) * P, :] for kc in range(3)], axis=1).astype(s2_np)
    wvroot_packed = np.concatenate(
        [wvrootT[kc * P:(kc + 1) * P, :] for kc in range(3)], axis=1).astype(s2_np)
    bias_t = np.ascontiguousarray(
        np.broadcast_to(np.asarray(b_scalar_root, np.float32), (P, H)))
    iota_t = np.ascontiguousarray(
        np.broadcast_to(np.arange(P, dtype=np.float32), (P, P)))
    ident_t = np.eye(P, dtype=np.float32).astype(s2_np)

    in_maps = []
    for core in range(N_CORES):
        base = core * NODES_PER_CORE
        xTc = xT[:, base:base + NODES_PER_CORE]  # [512, 1280]
        xTr = np.ascontiguousarray(
            xTc.reshape(4, P, NODES_PER_CORE).transpose(1, 0, 2)
               .reshape(P, 4 * NODES_PER_CORE)).astype(s2_np)
        in_maps.append({
            "xg": xg_full,
            "cols": np.ascontiguousarray(cols_arr[core]),
            "rr": np.ascontiguousarray(rr_arr[core]),
            "xt": xTr,
            "wsrel": wsrelT,
            "wsroot": wsrootT,
            "wvrel": wvrel_packed,
            "wvroot": wvroot_packed,
            "bias": bias_t,
            "iota": iota_t,
            "ident": ident_t,
        })

    nc = _get_program(T, cfg)
    kw = {}
    if PROFILE["on"]:
        kw = dict(trace=True, trace_cores=PROFILE["trace_cores"])
    res = run_bass_kernel_spmd(nc, in_maps, list(range(N_CORES)), **kw)
    PROFILE["last"] = res

    out_full = np.concatenate([res.results[i]["out"] for i in range(N_CORES)],
                              axis=0)
    return np.ascontiguousarray(
        out_full[:N_NODES].reshape(N_NODES, 4, H).astype(np.float32))


# revision 18
# speedup vs baseline: 1.6035x; 1.6035x over previous
"""Trainium2 Bass kernel for EquivariantGraphConv message passing.

Math (reference):
    scalar = x[:,0,:]; vector = x[:,1:,:].reshape(N, 3H)
    scalar_out = scalar @ Wsr.T + b + segsum(scalar[col] @ Wsrel.T, row)
    vector_out = vector @ Wvr.T + segsum(vector[col] @ Wvrel.T, row)

Key identity used: the edge transform is linear, so
    segsum(feat[col] @ W.T, row) == segsum(feat[col], row) @ W.T
We therefore aggregate the raw 512-dim node features per destination first
(16x fewer matmul FLOPs), then apply all four weight matrices per *node*.

Sharding: destinations are sharded across the 8 cores (1280 nodes each, in
10 chunks of 128).  Edges are sorted by destination on the host, so each
core only consumes its own edge shard and no cross-core reduction is
needed.  Each core gathers source features from a replicated padded node
table in DRAM with one big indirect DMA per chunk, builds one-hot
"selection" matrices on the vector engine (row_in_chunk == iota) and
matmul-accumulates P^T @ G into PSUM to realize the segment sum.
"""

import os
import sys

sys.path.insert(0, "/opt/trn_rl_repo")

import numpy as np
import ml_dtypes

import concourse.bass as bass
import concourse.mybir as mybir
import concourse.tile as tile
from concourse.bacc import Bacc
from concourse.bass_utils import run_bass_kernel_spmd

N_NODES = 10000
N_EDGES = 160000
H = 128
F = 4 * H            # 512 features per node (scalar 128 + vector 384)
P = 128              # partitions
NP_PAD = 10240       # padded node count (80 chunks of 128)
N_CORES = 8
NODES_PER_CORE = NP_PAD // N_CORES       # 1280
CHUNKS_PER_CORE = NODES_PER_CORE // P    # 10
N_CHUNKS = NP_PAD // P                   # 80
ZERO_ROW = N_NODES                       # padded zero row used by dummy edges
DEFAULT_T = 18                           # edge tiles per chunk (18*128 = 2304 cap)

# configuration: (gather/stage1 dtype, stage2 dtype); each of
# "bf16" | "f32" | "f32r".  f32r = fp32 storage, TF32-like matmul.
CFG = os.environ.get("BASS_GNN_CFG", "bf16,bf16")

# test.py hooks
PROFILE = {"on": False, "trace_cores": None, "last": None}

_prog_cache = {}


def _dt(name):
    return {
        "bf16": mybir.dt.bfloat16,
        "f32": mybir.dt.float32,
        "f32r": mybir.dt.float32,  # storage dtype; bitcast at matmul time
    }[name]


def _npdt(name):
    return {
        "bf16": ml_dtypes.bfloat16,
        "f32": np.float32,
        "f32r": np.float32,
    }[name]


def _build_program(T, cfg):
    """Build the (SPMD, per-core-identical) Bass program."""
    s1_name, s2_name = cfg
    s1_store = _dt(s1_name)
    s2_store = _dt(s2_name)

    def s1(ap):
        return ap.bitcast(mybir.dt.float32r) if s1_name == "f32r" else ap

    def s2(ap):
        return ap.bitcast(mybir.dt.float32r) if s2_name == "f32r" else ap

    nc = Bacc("TRN2", num_swdge_queues=4)
    f32 = mybir.dt.float32

    xg = nc.dram_tensor("xg", [NP_PAD, F], s1_store, kind="ExternalInput")
    # dma_gather is limited to ~1024 descriptors per instruction; split each
    # chunk's T*128 indices into NQ pieces of <= GQ indices.
    GQ = 1024
    NQ = (T * P + GQ - 1) // GQ
    WQ = GQ // 16  # idx columns per piece in the 16-partition wrapped layout
    cols = nc.dram_tensor("cols", [CHUNKS_PER_CORE, P, NQ * WQ], mybir.dt.int16,
                          kind="ExternalInput")
    rr = nc.dram_tensor("rr", [CHUNKS_PER_CORE, P, T], f32,
                        kind="ExternalInput")
    xt = nc.dram_tensor("xt", [P, 4 * NODES_PER_CORE], s2_store,
                        kind="ExternalInput")
    wsrel = nc.dram_tensor("wsrel", [P, H], s2_store, kind="ExternalInput")
    wsroot = nc.dram_tensor("wsroot", [P, H], s2_store, kind="ExternalInput")
    wvrel = nc.dram_tensor("wvrel", [P, 3 * 384], s2_store, kind="ExternalInput")
    wvroot = nc.dram_tensor("wvroot", [P, 3 * 384], s2_store, kind="ExternalInput")
    bias = nc.dram_tensor("bias", [P, H], f32, kind="ExternalInput")
    iota = nc.dram_tensor("iota", [P, P], f32, kind="ExternalInput")
    ident = nc.dram_tensor("ident", [P, P], s2_store, kind="ExternalInput")
    out = nc.dram_tensor("out", [NODES_PER_CORE, F], f32, kind="ExternalOutput")
    debug = os.environ.get("BASS_GNN_DEBUG", "0") == "1"
    if debug:
        dbg_agg = nc.dram_tensor("dbg_agg", [NODES_PER_CORE, F], f32,
                                 kind="ExternalOutput")
        dbg_g = nc.dram_tensor("dbg_g", [P, T * F], f32, kind="ExternalOutput")
        dbg_p = nc.dram_tensor("dbg_p", [P, T * P], f32, kind="ExternalOutput")

    with tile.TileContext(nc) as tc:
        with (
            tc.tile_pool(name="consts", bufs=1) as cpool,
            tc.tile_pool(name="edges", bufs=2) as epool,
            tc.tile_pool(name="work", bufs=2) as wpool,
            tc.tile_pool(name="pagg", bufs=2, space="PSUM") as pagg,
            tc.tile_pool(name="pmisc", bufs=2, space="PSUM") as pmisc,
        ):
            xt_sb = cpool.tile([P, 4 * NODES_PER_CORE], s2_store)
            nc.sync.dma_start(xt_sb[:], xt[:])
            wsrel_sb = cpool.tile([P, H], s2_store)
            nc.sync.dma_start(wsrel_sb[:], wsrel[:])
            wsroot_sb = cpool.tile([P, H], s2_store)
            nc.sync.dma_start(wsroot_sb[:], wsroot[:])
            wvrel_sb = cpool.tile([P, 3 * 384], s2_store)
            nc.sync.dma_start(wvrel_sb[:], wvrel[:])
            wvroot_sb = cpool.tile([P, 3 * 384], s2_store)
            nc.sync.dma_start(wvroot_sb[:], wvroot[:])
            bias_sb = cpool.tile([P, H], f32)
            nc.sync.dma_start(bias_sb[:], bias[:])
            iota_sb = cpool.tile([P, P], f32)
            nc.sync.dma_start(iota_sb[:], iota[:])
            ident_sb = cpool.tile([P, P], s2_store)
            nc.sync.dma_start(ident_sb[:], ident[:])

            for c in range(CHUNKS_PER_CORE):
                cols_sb = epool.tile([P, NQ * WQ], mybir.dt.int16, tag="cols")
                nc.sync.dma_start(cols_sb[:], cols[c])
                rr_sb = epool.tile([P, T], f32, tag="rr")
                nc.sync.dma_start(rr_sb[:], rr[c])

                # gather: edge i -> G[i % 128, i // 128, :] = xg[cols_flat[i], :]
                G = epool.tile([P, T * F], s1_store, tag="G")
                for q in range(NQ):
                    nidx = min(GQ, T * P - q * GQ)
                    nslots = nidx // P
                    nc.gpsimd.dma_gather(
                        G[:, q * (GQ // P) * F:
                             (q * (GQ // P) + nslots) * F]
                        .rearrange("p (t f) -> p t f", f=F),
                        xg[:],
                        cols_sb[:, q * WQ:(q + 1) * WQ],
                        nidx,
                        nidx,
                        F,
                        queue_num=(c * NQ + q) % 4,
                    )

                # one-hot P[p, t*128 + d] = (rr[p, t] == d)
                Pm = epool.tile([P, T * P], s1_store, tag="P")
                for t in range(T):
                    nc.vector.tensor_tensor(
                        out=Pm[:, t * P:(t + 1) * P],
                        in0=rr_sb[:, t:t + 1].to_broadcast([P, P]),
                        in1=iota_sb[:],
                        op=mybir.AluOpType.is_equal,
                    )

                # segment-sum: agg[d, f] = sum_t P_t^T @ G_t
                agg_ps = pagg.tile([P, F], f32, tag="agg")
                for t in range(T):
                    nc.tensor.matmul(
                        out=agg_ps[:],
                        lhsT=s1(Pm[:, t * P:(t + 1) * P]),
                        rhs=s1(G[:, t * F:(t + 1) * F]),
                        start=(t == 0),
                        stop=(t == T - 1),
                    )
                agg_sb = wpool.tile([P, F], s2_store, tag="aggsb")
                nc.vector.tensor_copy(agg_sb[:], agg_ps[:])
                if debug:
                    agg_f32_sb = wpool.tile([P, F], f32, tag="dbgagg")
                    nc.vector.tensor_copy(agg_f32_sb[:], agg_ps[:])
                    nc.sync.dma_start(dbg_agg[c * P:(c + 1) * P, :], agg_f32_sb[:])
                    if c == 0 and s1_store == f32:
                        nc.sync.dma_start(dbg_g[:], G[:])
                        nc.sync.dma_start(dbg_p[:], Pm[:])
                    elif c == 0:
                        g_f32_sb = wpool.tile([P, T * F], f32, tag="dbgg")
                        nc.vector.tensor_copy(g_f32_sb[:], G[:])
                        nc.sync.dma_start(dbg_g[:], g_f32_sb[:])
                        p_f32_sb = wpool.tile([P, T * P], f32, tag="dbgp")
                        nc.vector.tensor_copy(p_f32_sb[:], Pm[:])
                        nc.sync.dma_start(dbg_p[:], p_f32_sb[:])

                # transpose agg -> aggT[f, d] (4 PE transposes of 128x128)
                aggT_ps = pmisc.tile([P, F], s2_store, tag="aggT")
                for fc in range(4):
                    nc.tensor.transpose(
                        out=s2(aggT_ps[:, fc * P:(fc + 1) * P]),
                        in_=s2(agg_sb[:, fc * P:(fc + 1) * P]),
                        identity=s2(ident_sb[:]),
                    )
                aggT_sb = wpool.tile([P, F], s2_store, tag="aggTsb")
                nc.vector.tensor_copy(aggT_sb[:], aggT_ps[:])

                # stage 2: out[d, :128]  = agg_s @ WsrelT + x_s @ WsrootT (+bias)
                #          out[d, 128:]  = agg_v @ WvrelT + x_v @ WvrootT
                osv_ps = pmisc.tile([P, F], f32, tag="osv")
                nc.tensor.matmul(out=osv_ps[:, 0:H],
                                 lhsT=s2(aggT_sb[:, 0:P]), rhs=s2(wsrel_sb[:]),
                                 start=True, stop=False)
                nc.tensor.matmul(out=osv_ps[:, 0:H],
                                 lhsT=s2(xt_sb[:, c * P:(c + 1) * P]),
                                 rhs=s2(wsroot_sb[:]),
                                 start=False, stop=True)
                for kc in range(3):
                    nc.tensor.matmul(
                        out=osv_ps[:, H:F],
                        lhsT=s2(aggT_sb[:, (1 + kc) * P:(2 + kc) * P]),
                        rhs=s2(wvrel_sb[:, kc * 384:(kc + 1) * 384]),
                        start=(kc == 0), stop=False)
                for kc in range(3):
                    nc.tensor.matmul(
                        out=osv_ps[:, H:F],
                        lhsT=s2(xt_sb[:, (1 + kc) * NODES_PER_CORE + c * P:
                                      (1 + kc) * NODES_PER_CORE + (c + 1) * P]),
                        rhs=s2(wvroot_sb[:, kc * 384:(kc + 1) * 384]),
                        start=False, stop=(kc == 2))

                out_sb = wpool.tile([P, F], f32, tag="outsb")
                nc.vector.tensor_add(out_sb[:, 0:H], osv_ps[:, 0:H], bias_sb[:])
                nc.vector.tensor_copy(out_sb[:, H:F], osv_ps[:, H:F])
                nc.sync.dma_start(out[c * P:(c + 1) * P, :], out_sb[:])

    nc.finalize()
    return nc


def _get_program(T, cfg):
    key = (T, cfg)
    if key not in _prog_cache:
        _prog_cache[key] = _build_program(T, cfg)
    return _prog_cache[key]


def kernel(x, edge_index, W_scalar_rel, W_scalar_root, b_scalar_root,
           W_vector_rel, W_vector_root):
    cfg = tuple(CFG.split(","))
    s1_np = _npdt(cfg[0])
    s2_np = _npdt(cfg[1])

    x = np.asarray(x, dtype=np.float32)
    n = x.shape[0]
    assert n == N_NODES, x.shape
    row = np.asarray(edge_index[0], dtype=np.int64)
    col = np.asarray(edge_index[1], dtype=np.int64)

    # ---- host-side shard construction (sort edges by destination) ----
    order = np.argsort(row, kind="stable")
    row_s = row[order]
    col_s = col[order]
    bounds = np.searchsorted(row_s, np.arange(0, NP_PAD + 1, P))
    counts = np.diff(bounds)
    T = max(DEFAULT_T, int(np.ceil(counts.max() / P)))

    cap = T * P
    cols_pad = np.full((N_CHUNKS, cap), ZERO_ROW, dtype=np.int16)
    rr_pad = np.zeros((N_CHUNKS, cap), dtype=np.float32)
    for g in range(N_CHUNKS):
        s, e = bounds[g], bounds[g + 1]
        m = e - s
        if m:
            cols_pad[g, :m] = col_s[s:e]
            rr_pad[g, :m] = (row_s[s:e] - g * P).astype(np.float32)
    # dma_gather: flat edge i -> partition i % 128, tile-slot i // 128.
    # Each chunk's indices are split into NQ pieces of <= 1024; within a
    # piece, idx element j lives at wrapped position [j % 16, j // 16],
    # and the 16-row block is replicated across all 128 partitions
    # (the tx/rx Q7 cores each read their own 16-partition copy).
    GQ = 1024
    NQ = (cap + GQ - 1) // GQ
    WQ = GQ // 16
    cols_q = np.zeros((N_CHUNKS, NQ, GQ), dtype=np.int16)
    cols_q.reshape(N_CHUNKS, NQ * GQ)[:, :cap] = cols_pad
    wrap = cols_q.reshape(N_CHUNKS, NQ, WQ, 16).transpose(0, 1, 3, 2)  # [.., 16, WQ]
    cols_wrapped = np.tile(wrap, (1, 1, 8, 1))  # [N_CHUNKS, NQ, 128, WQ]
    cols_flat = cols_wrapped.transpose(0, 2, 1, 3).reshape(N_CHUNKS, P, NQ * WQ)
    cols_arr = np.ascontiguousarray(cols_flat).reshape(
        N_CORES, CHUNKS_PER_CORE, P, NQ * WQ)
    # rr for edge i goes to [i % 128, i // 128]
    rr_arr = rr_pad.reshape(N_CHUNKS, T, P).transpose(0, 2, 1)
    rr_arr = np.ascontiguousarray(rr_arr).reshape(N_CORES, CHUNKS_PER_CORE, P, T)

    x_flat = np.zeros((NP_PAD, F), dtype=np.float32)
    x_flat[:n] = x.reshape(n, F)
    xg_full = np.ascontiguousarray(x_flat.astype(s1_np))

    xT = x_flat.T  # [512, 10240], exact f32 for the root transform

    wsrelT = np.ascontiguousarray(np.asarray(W_scalar_rel, np.float32).T).astype(s2_np)
    wsrootT = np.ascontiguousarray(np.asarray(W_scalar_root, np.float32).T).astype(s2_np)
    wvrelT = np.ascontiguousarray(np.asarray(W_vector_rel, np.float32).T)
    wvrootT = np.ascontiguousarray(np.asarray(W_vector_root, np.float32).T)
    wvrel_packed = np.concatenate(
        [wvrelT[kc * P:(kc + 1) * P, :] for kc in range(3)], axis=1).astype(s2_np)
    wvroot_packed = np.concatenate(
        [wvrootT[kc * P:(kc + 1) * P, :] for kc in range(3)], axis=1).astype(s2_np)
    bias_t = np.ascontiguousarray(
        np.broadcast_to(np.asarray(b_scalar_root, np.float32), (P, H)))
    iota_t = np.ascontiguousarray(
        np.broadcast_to(np.arange(P, dtype=np.float32), (P, P)))
    ident_t = np.eye(P, dtype=np.float32).astype(s2_np)

    in_maps = []
    for core in range(N_CORES):
        base = core * NODES_PER_CORE
        xTc = xT[:, base:base + NODES_PER_CORE]  # [512, 1280]
        xTr = np.ascontiguousarray(
            xTc.reshape(4, P, NODES_PER_CORE).transpose(1, 0, 2)
               .reshape(P, 4 * NODES_PER_CORE)).astype(s2_np)
        in_maps.append({
            "xg": xg_full,
            "cols": np.ascontiguousarray(cols_arr[core]),
            "rr": np.ascontiguousarray(rr_arr[core]),
            "xt": xTr,
            "wsrel": wsrelT,
            "wsroot": wsrootT,
            "wvrel": wvrel_packed,
            "wvroot": wvroot_packed,
            "bias": bias_t,
            "iota": iota_t,
            "ident": ident_t,
        })

    nc = _get_program(T, cfg)
    kw = {}
    if PROFILE["on"]:
        kw = dict(trace=True, trace_cores=PROFILE["trace_cores"])
    res = run_bass_kernel_spmd(nc, in_maps, list(range(N_CORES)), **kw)
    PROFILE["last"] = res

    out_full = np.concatenate([res.results[i]["out"] for i in range(N_CORES)],
                              axis=0)
    return np.ascontiguousarray(
        out_full[:N_NODES].reshape(N_NODES, 4, H).astype(np.float32))


# revision 21
# speedup vs baseline: 1.9006x; 1.1853x over previous
"""Trainium2 Bass kernel for EquivariantGraphConv message passing.

Math (reference):
    scalar = x[:,0,:]; vector = x[:,1:,:].reshape(N, 3H)
    scalar_out = scalar @ Wsr.T + b + segsum(scalar[col] @ Wsrel.T, row)
    vector_out = vector @ Wvr.T + segsum(vector[col] @ Wvrel.T, row)

Key identity used: the edge transform is linear, so
    segsum(feat[col] @ W.T, row) == segsum(feat[col], row) @ W.T
We therefore aggregate the raw 512-dim node features per destination first
(16x fewer matmul FLOPs), then apply all four weight matrices per *node*.

Sharding: destinations are sharded across the 8 cores (1280 nodes each, in
10 chunks of 128).  Edges are sorted by destination on the host, so each
core only consumes its own edge shard and no cross-core reduction is
needed.  Each core gathers source features from a replicated padded node
table in DRAM with one big indirect DMA per chunk, builds one-hot
"selection" matrices on the vector engine (row_in_chunk == iota) and
matmul-accumulates P^T @ G into PSUM to realize the segment sum.
"""

import os
import sys

sys.path.insert(0, "/opt/trn_rl_repo")

import numpy as np
import ml_dtypes

import concourse.bass as bass
import concourse.mybir as mybir
import concourse.tile as tile
from concourse.bacc import Bacc
from concourse.bass_utils import run_bass_kernel_spmd

N_NODES = 10000
N_EDGES = 160000
H = 128
F = 4 * H            # 512 features per node (scalar 128 + vector 384)
P = 128              # partitions
NP_PAD = 10240       # padded node count (80 chunks of 128)
N_CORES = 8
NODES_PER_CORE = NP_PAD // N_CORES       # 1280
CHUNKS_PER_CORE = NODES_PER_CORE // P    # 10
N_CHUNKS = NP_PAD // P                   # 80
ZERO_ROW = N_NODES                       # padded zero row used by dummy edges
DEFAULT_T = 17                           # edge tiles per chunk (17*128 = 2176 cap)

# configuration: (gather/stage1 dtype, stage2 dtype); each of
# "bf16" | "f32" | "f32r".  f32r = fp32 storage, TF32-like matmul.
CFG = os.environ.get("BASS_GNN_CFG", "bf16,bf16")

# test.py hooks
PROFILE = {"on": False, "trace_cores": None, "last": None}

_prog_cache = {}


def _dt(name):
    return {
        "bf16": mybir.dt.bfloat16,
        "f32": mybir.dt.float32,
        "f32r": mybir.dt.float32,  # storage dtype; bitcast at matmul time
    }[name]


def _npdt(name):
    return {
        "bf16": ml_dtypes.bfloat16,
        "f32": np.float32,
        "f32r": np.float32,
    }[name]


def _build_program(T, cfg):
    """Build the (SPMD, per-core-identical) Bass program."""
    s1_name, s2_name = cfg
    s1_store = _dt(s1_name)
    s2_store = _dt(s2_name)

    def s1(ap):
        return ap.bitcast(mybir.dt.float32r) if s1_name == "f32r" else ap

    def s2(ap):
        return ap.bitcast(mybir.dt.float32r) if s2_name == "f32r" else ap

    nc = Bacc("TRN2", num_swdge_queues=4)
    f32 = mybir.dt.float32

    xg = nc.dram_tensor("xg", [NP_PAD, F], s1_store, kind="ExternalInput")
    # dma_gather is limited to ~1024 descriptors per instruction; split each
    # chunk's T*128 indices into NQ pieces of <= GQ indices.
    GQ = 1024
    NQ = (T * P + GQ - 1) // GQ
    WQ = GQ // 16  # idx columns per piece in the 16-partition wrapped layout
    cols = nc.dram_tensor("cols", [CHUNKS_PER_CORE, P, NQ * WQ], mybir.dt.int16,
                          kind="ExternalInput")
    rr = nc.dram_tensor("rr", [CHUNKS_PER_CORE, P, T], f32,
                        kind="ExternalInput")
    xt = nc.dram_tensor("xt", [P, 4 * NODES_PER_CORE], s2_store,
                        kind="ExternalInput")
    wsrel = nc.dram_tensor("wsrel", [P, H], s2_store, kind="ExternalInput")
    wsroot = nc.dram_tensor("wsroot", [P, H], s2_store, kind="ExternalInput")
    wvrel = nc.dram_tensor("wvrel", [P, 3 * 384], s2_store, kind="ExternalInput")
    wvroot = nc.dram_tensor("wvroot", [P, 3 * 384], s2_store, kind="ExternalInput")
    bias = nc.dram_tensor("bias", [P, H], f32, kind="ExternalInput")
    iota = nc.dram_tensor("iota", [P, P], f32, kind="ExternalInput")
    ident = nc.dram_tensor("ident", [P, P], s2_store, kind="ExternalInput")
    out = nc.dram_tensor("out", [NODES_PER_CORE, F], f32, kind="ExternalOutput")
    debug = os.environ.get("BASS_GNN_DEBUG", "0") == "1"
    if debug:
        dbg_agg = nc.dram_tensor("dbg_agg", [NODES_PER_CORE, F], f32,
                                 kind="ExternalOutput")
        dbg_g = nc.dram_tensor("dbg_g", [P, T * F], f32, kind="ExternalOutput")
        dbg_p = nc.dram_tensor("dbg_p", [P, T * P], f32, kind="ExternalOutput")

    with tile.TileContext(nc) as tc:
        with (
            tc.tile_pool(name="consts", bufs=1) as cpool,
            tc.tile_pool(name="edges", bufs=3) as epool,
            tc.tile_pool(name="gbuf", bufs=4) as gpool,
            tc.tile_pool(name="work", bufs=2) as wpool,
            tc.tile_pool(name="pagg", bufs=3, space="PSUM") as pagg,
            tc.tile_pool(name="pmisc", bufs=2, space="PSUM") as pmisc,
        ):
            # constants go on the Scalar HWDGE queue so the first chunk's
            # index DMAs (sync queue) aren't stuck behind the 2.6MB xt load
            xt_sb = cpool.tile([P, 4 * NODES_PER_CORE], s2_store)
            nc.scalar.dma_start(xt_sb[:], xt[:])
            wsrel_sb = cpool.tile([P, H], s2_store)
            nc.scalar.dma_start(wsrel_sb[:], wsrel[:])
            wsroot_sb = cpool.tile([P, H], s2_store)
            nc.scalar.dma_start(wsroot_sb[:], wsroot[:])
            wvrel_sb = cpool.tile([P, 3 * 384], s2_store)
            nc.scalar.dma_start(wvrel_sb[:], wvrel[:])
            wvroot_sb = cpool.tile([P, 3 * 384], s2_store)
            nc.scalar.dma_start(wvroot_sb[:], wvroot[:])
            bias_sb = cpool.tile([P, H], f32)
            nc.scalar.dma_start(bias_sb[:], bias[:])
            iota_sb = cpool.tile([P, P], f32)
            nc.scalar.dma_start(iota_sb[:], iota[:])
            ident_sb = cpool.tile([P, P], s2_store)
            nc.scalar.dma_start(ident_sb[:], ident[:])

            for c in range(CHUNKS_PER_CORE):
                cols_sb = epool.tile([P, NQ * WQ], mybir.dt.int16, tag="cols")
                nc.sync.dma_start(cols_sb[:], cols[c])
                rr_sb = epool.tile([P, T], f32, tag="rr")
                nc.sync.dma_start(rr_sb[:], rr[c])

                # gather: edge i -> G[i % 128, i // 128, :] = xg[cols_flat[i], :]
                G = gpool.tile([P, T * F], s1_store, tag="G")
                for q in range(NQ):
                    nidx = min(GQ, T * P - q * GQ)
                    nslots = nidx // P
                    nc.gpsimd.dma_gather(
                        G[:, q * (GQ // P) * F:
                             (q * (GQ // P) + nslots) * F]
                        .rearrange("p (t f) -> p t f", f=F),
                        xg[:],
                        cols_sb[:, q * WQ:(q + 1) * WQ],
                        nidx,
                        nidx,
                        F,
                        queue_num=(c * NQ + q) % 4,
                    )

                # one-hot P[p, t*128 + d] = (rr[p, t] == d)
                Pm = epool.tile([P, T * P], s1_store, tag="P")
                for t in range(T):
                    nc.vector.tensor_tensor(
                        out=Pm[:, t * P:(t + 1) * P],
                        in0=rr_sb[:, t:t + 1].to_broadcast([P, P]),
                        in1=iota_sb[:],
                        op=mybir.AluOpType.is_equal,
                    )

                # segment-sum: agg[d, f] = sum_t P_t^T @ G_t
                agg_ps = pagg.tile([P, F], f32, tag="agg")
                for t in range(T):
                    nc.tensor.matmul(
                        out=agg_ps[:],
                        lhsT=s1(Pm[:, t * P:(t + 1) * P]),
                        rhs=s1(G[:, t * F:(t + 1) * F]),
                        start=(t == 0),
                        stop=(t == T - 1),
                    )
                agg_sb = wpool.tile([P, F], s2_store, tag="aggsb")
                nc.vector.tensor_copy(agg_sb[:], agg_ps[:])
                if debug:
                    agg_f32_sb = wpool.tile([P, F], f32, tag="dbgagg")
                    nc.vector.tensor_copy(agg_f32_sb[:], agg_ps[:])
                    nc.sync.dma_start(dbg_agg[c * P:(c + 1) * P, :], agg_f32_sb[:])
                    if c == 0 and s1_store == f32:
                        nc.sync.dma_start(dbg_g[:], G[:])
                        nc.sync.dma_start(dbg_p[:], Pm[:])
                    elif c == 0:
                        g_f32_sb = wpool.tile([P, T * F], f32, tag="dbgg")
                        nc.vector.tensor_copy(g_f32_sb[:], G[:])
                        nc.sync.dma_start(dbg_g[:], g_f32_sb[:])
                        p_f32_sb = wpool.tile([P, T * P], f32, tag="dbgp")
                        nc.vector.tensor_copy(p_f32_sb[:], Pm[:])
                        nc.sync.dma_start(dbg_p[:], p_f32_sb[:])

                # transpose agg -> aggT[f, d] (4 PE transposes of 128x128)
                aggT_ps = pmisc.tile([P, F], s2_store, tag="aggT")
                for fc in range(4):
                    nc.tensor.transpose(
                        out=s2(aggT_ps[:, fc * P:(fc + 1) * P]),
                        in_=s2(agg_sb[:, fc * P:(fc + 1) * P]),
                        identity=s2(ident_sb[:]),
                    )
                aggT_sb = wpool.tile([P, F], s2_store, tag="aggTsb")
                nc.vector.tensor_copy(aggT_sb[:], aggT_ps[:])

                # stage 2: out[d, :128]  = agg_s @ WsrelT + x_s @ WsrootT (+bias)
                #          out[d, 128:]  = agg_v @ WvrelT + x_v @ WvrootT
                osv_ps = pmisc.tile([P, F], f32, tag="osv")
                nc.tensor.matmul(out=osv_ps[:, 0:H],
                                 lhsT=s2(aggT_sb[:, 0:P]), rhs=s2(wsrel_sb[:]),
                                 start=True, stop=False)
                nc.tensor.matmul(out=osv_ps[:, 0:H],
                                 lhsT=s2(xt_sb[:, c * P:(c + 1) * P]),
                                 rhs=s2(wsroot_sb[:]),
                                 start=False, stop=True)
                for kc in range(3):
                    nc.tensor.matmul(
                        out=osv_ps[:, H:F],
                        lhsT=s2(aggT_sb[:, (1 + kc) * P:(2 + kc) * P]),
                        rhs=s2(wvrel_sb[:, kc * 384:(kc + 1) * 384]),
                        start=(kc == 0), stop=False)
                for kc in range(3):
                    nc.tensor.matmul(
                        out=osv_ps[:, H:F],
                        lhsT=s2(xt_sb[:, (1 + kc) * NODES_PER_CORE + c * P:
                                      (1 + kc) * NODES_PER_CORE + (c + 1) * P]),
                        rhs=s2(wvroot_sb[:, kc * 384:(kc + 1) * 384]),
                        start=False, stop=(kc == 2))

                out_sb = wpool.tile([P, F], f32, tag="outsb")
                nc.vector.tensor_add(out_sb[:, 0:H], osv_ps[:, 0:H], bias_sb[:])
                nc.vector.tensor_copy(out_sb[:, H:F], osv_ps[:, H:F])
                nc.sync.dma_start(out[c * P:(c + 1) * P, :], out_sb[:])

    nc.finalize()
    return nc


def _get_program(T, cfg):
    key = (T, cfg)
    if key not in _prog_cache:
        _prog_cache[key] = _build_program(T, cfg)
    return _prog_cache[key]


def kernel(x, edge_index, W_scalar_rel, W_scalar_root, b_scalar_root,
           W_vector_rel, W_vector_root):
    cfg = tuple(CFG.split(","))
    s1_np = _npdt(cfg[0])
    s2_np = _npdt(cfg[1])

    x = np.asarray(x, dtype=np.float32)
    n = x.shape[0]
    assert n == N_NODES, x.shape
    row = np.asarray(edge_index[0], dtype=np.int64)
    col = np.asarray(edge_index[1], dtype=np.int64)

    # ---- host-side shard construction (sort edges by destination) ----
    order = np.argsort(row, kind="stable")
    row_s = row[order]
    col_s = col[order]
    bounds = np.searchsorted(row_s, np.arange(0, NP_PAD + 1, P))
    counts = np.diff(bounds)
    T = max(DEFAULT_T, int(np.ceil(counts.max() / P)))

    cap = T * P
    cols_pad = np.full((N_CHUNKS, cap), ZERO_ROW, dtype=np.int16)
    rr_pad = np.zeros((N_CHUNKS, cap), dtype=np.float32)
    for g in range(N_CHUNKS):
        s, e = bounds[g], bounds[g + 1]
        m = e - s
        if m:
            cols_pad[g, :m] = col_s[s:e]
            rr_pad[g, :m] = (row_s[s:e] - g * P).astype(np.float32)
    # dma_gather: flat edge i -> partition i % 128, tile-slot i // 128.
    # Each chunk's indices are split into NQ pieces of <= 1024; within a
    # piece, idx element j lives at wrapped position [j % 16, j // 16],
    # and the 16-row block is replicated across all 128 partitions
    # (the tx/rx Q7 cores each read their own 16-partition copy).
    GQ = 1024
    NQ = (cap + GQ - 1) // GQ
    WQ = GQ // 16
    cols_q = np.zeros((N_CHUNKS, NQ, GQ), dtype=np.int16)
    cols_q.reshape(N_CHUNKS, NQ * GQ)[:, :cap] = cols_pad
    wrap = cols_q.reshape(N_CHUNKS, NQ, WQ, 16).transpose(0, 1, 3, 2)  # [.., 16, WQ]
    cols_wrapped = np.tile(wrap, (1, 1, 8, 1))  # [N_CHUNKS, NQ, 128, WQ]
    cols_flat = cols_wrapped.transpose(0, 2, 1, 3).reshape(N_CHUNKS, P, NQ * WQ)
    cols_arr = np.ascontiguousarray(cols_flat).reshape(
        N_CORES, CHUNKS_PER_CORE, P, NQ * WQ)
    # rr for edge i goes to [i % 128, i // 128]
    rr_arr = rr_pad.reshape(N_CHUNKS, T, P).transpose(0, 2, 1)
    rr_arr = np.ascontiguousarray(rr_arr).reshape(N_CORES, CHUNKS_PER_CORE, P, T)

    x_flat = np.zeros((NP_PAD, F), dtype=np.float32)
    x_flat[:n] = x.reshape(n, F)
    xg_full = np.ascontiguousarray(x_flat.astype(s1_np))

    xT = x_flat.T  # [512, 10240], exact f32 for the root transform

    wsrelT = np.ascontiguousarray(np.asarray(W_scalar_rel, np.float32).T).astype(s2_np)
    wsrootT = np.ascontiguousarray(np.asarray(W_scalar_root, np.float32).T).astype(s2_np)
    wvrelT = np.ascontiguousarray(np.asarray(W_vector_rel, np.float32).T)
    wvrootT = np.ascontiguousarray(np.asarray(W_vector_root, np.float32).T)
    wvrel_packed = np.concatenate(
        [wvrelT[kc * P:(kc + 1) * P, :] for kc in range(3)], axis=1).astype(s2_np)
    wvroot_packed = np.concatenate(
        [wvrootT[kc * P:(kc + 1) * P, :] for kc in range(3)], axis=1).astype(s2_np)
    bias_t = np.ascontiguousarray(
        np.broadcast_to(np.asarray(b_scalar_root, np.float32), (P, H)))
    iota_t = np.ascontiguousarray(
        np.broadcast_to(np.arange(P, dtype=np.float32), (P, P)))
    ident_t = np.eye(P, dtype=np.float32).astype(s2_np)

    in_maps = []
    for core in range(N_CORES):
        base = core * NODES_PER_CORE
        xTc = xT[:, base:base + NODES_PER_CORE]  # [512, 1280]
        xTr = np.ascontiguousarray(
            xTc.reshape(4, P, NODES_PER_CORE).transpose(1, 0, 2)
               .reshape(P, 4 * NODES_PER_CORE)).astype(s2_np)
        in_maps.append({
            "xg": xg_full,
            "cols": np.ascontiguousarray(cols_arr[core]),
            "rr": np.ascontiguousarray(rr_arr[core]),
            "xt": xTr,
            "wsrel": wsrelT,
            "wsroot": wsrootT,
            "wvrel": wvrel_packed,
            "wvroot": wvroot_packed,
            "bias": bias_t,
            "iota": iota_t,
            "ident": ident_t,
        })

    nc = _get_program(T, cfg)
    kw = {}
    if PROFILE["on"]:
        kw = dict(trace=True, trace_cores=PROFILE["trace_cores"])
    res = run_bass_kernel_spmd(nc, in_maps, list(range(N_CORES)), **kw)
    PROFILE["last"] = res

    out_full = np.concatenate([res.results[i]["out"] for i in range(N_CORES)],
                              axis=0)
    return np.ascontiguousarray(
        out_full[:N_NODES].reshape(N_NODES, 4, H).astype(np.float32))


# revision 26
# speedup vs baseline: 1.9750x; 1.0391x over previous
"""Trainium2 Bass kernel for EquivariantGraphConv message passing.

Math (reference):
    scalar = x[:,0,:]; vector = x[:,1:,:].reshape(N, 3H)
    scalar_out = scalar @ Wsr.T + b + segsum(scalar[col] @ Wsrel.T, row)
    vector_out = vector @ Wvr.T + segsum(vector[col] @ Wvrel.T, row)

Key identity used: the edge transform is linear, so
    segsum(feat[col] @ W.T, row) == segsum(feat[col], row) @ W.T
We therefore aggregate the raw 512-dim node features per destination first
(16x fewer matmul FLOPs), then apply all four weight matrices per *node*.

Sharding: destinations are sharded across the 8 cores (1280 nodes each, in
10 chunks of 128).  Edges are sorted by destination on the host, so each
core only consumes its own edge shard and no cross-core reduction is
needed.  Each core gathers source features from a replicated padded node
table in DRAM with one big indirect DMA per chunk, builds one-hot
"selection" matrices on the vector engine (row_in_chunk == iota) and
matmul-accumulates P^T @ G into PSUM to realize the segment sum.
"""

import os
import sys

sys.path.insert(0, "/opt/trn_rl_repo")

import numpy as np
import ml_dtypes

import concourse.bass as bass
import concourse.mybir as mybir
import concourse.tile as tile
from concourse.bacc import Bacc
from concourse.bass_utils import run_bass_kernel_spmd

N_NODES = 10000
N_EDGES = 160000
H = 128
F = 4 * H            # 512 features per node (scalar 128 + vector 384)
P = 128              # partitions
NP_PAD = 10240       # padded node count (80 chunks of 128)
N_CORES = 8
NODES_PER_CORE = NP_PAD // N_CORES       # 1280
CHUNKS_PER_CORE = NODES_PER_CORE // P    # 10
N_CHUNKS = NP_PAD // P                   # 80
ZERO_ROW = N_NODES                       # padded zero row used by dummy edges
DEFAULT_T = 17                           # edge tiles per chunk (17*128 = 2176 cap)

# configuration: (gather/stage1 dtype, stage2 dtype); each of
# "bf16" | "f32" | "f32r".  f32r = fp32 storage, TF32-like matmul.
CFG = os.environ.get("BASS_GNN_CFG", "bf16,bf16")

# test.py hooks
PROFILE = {"on": False, "trace_cores": None, "last": None}

_prog_cache = {}


def _dt(name):
    return {
        "bf16": mybir.dt.bfloat16,
        "f32": mybir.dt.float32,
        "f32r": mybir.dt.float32,  # storage dtype; bitcast at matmul time
    }[name]


def _npdt(name):
    return {
        "bf16": ml_dtypes.bfloat16,
        "f32": np.float32,
        "f32r": np.float32,
    }[name]


def _build_program(T, cfg):
    """Build the (SPMD, per-core-identical) Bass program."""
    s1_name, s2_name = cfg
    s1_store = _dt(s1_name)
    s2_store = _dt(s2_name)

    def s1(ap):
        return ap.bitcast(mybir.dt.float32r) if s1_name == "f32r" else ap

    def s2(ap):
        return ap.bitcast(mybir.dt.float32r) if s2_name == "f32r" else ap

    nc = Bacc("TRN2", num_swdge_queues=4)
    f32 = mybir.dt.float32

    xg = nc.dram_tensor("xg", [NP_PAD, F], s1_store, kind="ExternalInput")
    # dma_gather is limited to ~1024 descriptors per instruction; split each
    # chunk's T*128 indices into NQ pieces of <= GQ indices.
    GQ = 1024
    NQ = (T * P + GQ - 1) // GQ
    WQ = GQ // 16  # idx columns per piece in the 16-partition wrapped layout
    cols = nc.dram_tensor("cols", [CHUNKS_PER_CORE, P, NQ * WQ], mybir.dt.int16,
                          kind="ExternalInput")
    rr = nc.dram_tensor("rr", [CHUNKS_PER_CORE, P, T], f32,
                        kind="ExternalInput")
    xt = nc.dram_tensor("xt", [P, 4 * NODES_PER_CORE], s2_store,
                        kind="ExternalInput")
    wsrel = nc.dram_tensor("wsrel", [P, H], s2_store, kind="ExternalInput")
    wsroot = nc.dram_tensor("wsroot", [P, H], s2_store, kind="ExternalInput")
    wvrel = nc.dram_tensor("wvrel", [P, 3 * 384], s2_store, kind="ExternalInput")
    wvroot = nc.dram_tensor("wvroot", [P, 3 * 384], s2_store, kind="ExternalInput")
    bias = nc.dram_tensor("bias", [P, H], f32, kind="ExternalInput")
    iota = nc.dram_tensor("iota", [P, P], f32, kind="ExternalInput")
    ident = nc.dram_tensor("ident", [P, P], s2_store, kind="ExternalInput")
    out = nc.dram_tensor("out", [NODES_PER_CORE, F], f32, kind="ExternalOutput")
    debug = os.environ.get("BASS_GNN_DEBUG", "0") == "1"
    if debug:
        dbg_agg = nc.dram_tensor("dbg_agg", [NODES_PER_CORE, F], f32,
                                 kind="ExternalOutput")
        dbg_g = nc.dram_tensor("dbg_g", [P, T * F], f32, kind="ExternalOutput")
        dbg_p = nc.dram_tensor("dbg_p", [P, T * P], f32, kind="ExternalOutput")

    with tile.TileContext(nc) as tc:
        with (
            tc.tile_pool(name="consts", bufs=1) as cpool,
            tc.tile_pool(name="edges", bufs=3) as epool,
            tc.tile_pool(name="gbuf", bufs=4) as gpool,
            tc.tile_pool(name="work", bufs=2) as wpool,
            tc.tile_pool(name="pagg", bufs=3, space="PSUM") as pagg,
            tc.tile_pool(name="pmisc", bufs=2, space="PSUM") as pmisc,
        ):
            # constants go on the Scalar HWDGE queue so the first chunk's
            # index DMAs (sync queue) aren't stuck behind the 2.6MB xt load
            xt_sb = cpool.tile([P, 4 * NODES_PER_CORE], s2_store)
            nc.scalar.dma_start(xt_sb[:], xt[:])
            wsrel_sb = cpool.tile([P, H], s2_store)
            nc.scalar.dma_start(wsrel_sb[:], wsrel[:])
            wsroot_sb = cpool.tile([P, H], s2_store)
            nc.scalar.dma_start(wsroot_sb[:], wsroot[:])
            wvrel_sb = cpool.tile([P, 3 * 384], s2_store)
            nc.scalar.dma_start(wvrel_sb[:], wvrel[:])
            wvroot_sb = cpool.tile([P, 3 * 384], s2_store)
            nc.scalar.dma_start(wvroot_sb[:], wvroot[:])
            bias_sb = cpool.tile([P, H], f32)
            nc.scalar.dma_start(bias_sb[:], bias[:])
            iota_sb = cpool.tile([P, P], f32)
            nc.scalar.dma_start(iota_sb[:], iota[:])
            ident_sb = cpool.tile([P, P], s2_store)
            nc.scalar.dma_start(ident_sb[:], ident[:])

            for c in range(CHUNKS_PER_CORE):
                cols_sb = epool.tile([P, NQ * WQ], mybir.dt.int16, tag="cols")
                nc.sync.dma_start(cols_sb[:], cols[c])
                rr_sb = epool.tile([P, T], f32, tag="rr")
                nc.sync.dma_start(rr_sb[:], rr[c])

                # gather: edge i -> G[i % 128, i // 128, :] = xg[cols_flat[i], :]
                G = gpool.tile([P, T * F], s1_store, tag="G")
                for q in range(NQ):
                    nidx = min(GQ, T * P - q * GQ)
                    nslots = nidx // P
                    nc.gpsimd.dma_gather(
                        G[:, q * (GQ // P) * F:
                             (q * (GQ // P) + nslots) * F]
                        .rearrange("p (t f) -> p t f", f=F),
                        xg[:],
                        cols_sb[:, q * WQ:(q + 1) * WQ],
                        nidx,
                        nidx,
                        F,
                        queue_num=(c * NQ + q) % 4,
                    )

                # one-hot P[p, t*128 + d] = (rr[p, t] == d)
                Pm = epool.tile([P, T * P], s1_store, tag="P")
                for t in range(T):
                    nc.vector.tensor_tensor(
                        out=Pm[:, t * P:(t + 1) * P],
                        in0=rr_sb[:, t:t + 1].to_broadcast([P, P]),
                        in1=iota_sb[:],
                        op=mybir.AluOpType.is_equal,
                    )

                # segment-sum: agg[d, f] = sum_t P_t^T @ G_t
                agg_ps = pagg.tile([P, F], f32, tag="agg")
                for t in range(T):
                    nc.tensor.matmul(
                        out=agg_ps[:],
                        lhsT=s1(Pm[:, t * P:(t + 1) * P]),
                        rhs=s1(G[:, t * F:(t + 1) * F]),
                        start=(t == 0),
                        stop=(t == T - 1),
                    )
                agg_sb = wpool.tile([P, F], s2_store, tag="aggsb")
                nc.vector.tensor_copy(agg_sb[:], agg_ps[:])
                if debug:
                    agg_f32_sb = wpool.tile([P, F], f32, tag="dbgagg")
                    nc.vector.tensor_copy(agg_f32_sb[:], agg_ps[:])
                    nc.sync.dma_start(dbg_agg[c * P:(c + 1) * P, :], agg_f32_sb[:])
                    if c == 0 and s1_store == f32:
                        nc.sync.dma_start(dbg_g[:], G[:])
                        nc.sync.dma_start(dbg_p[:], Pm[:])
                    elif c == 0:
                        g_f32_sb = wpool.tile([P, T * F], f32, tag="dbgg")
                        nc.vector.tensor_copy(g_f32_sb[:], G[:])
                        nc.sync.dma_start(dbg_g[:], g_f32_sb[:])
                        p_f32_sb = wpool.tile([P, T * P], f32, tag="dbgp")
                        nc.vector.tensor_copy(p_f32_sb[:], Pm[:])
                        nc.sync.dma_start(dbg_p[:], p_f32_sb[:])

                # transpose agg -> aggT[f, d] (4 PE transposes of 128x128)
                aggT_ps = pmisc.tile([P, F], s2_store, tag="aggT")
                for fc in range(4):
                    nc.tensor.transpose(
                        out=s2(aggT_ps[:, fc * P:(fc + 1) * P]),
                        in_=s2(agg_sb[:, fc * P:(fc + 1) * P]),
                        identity=s2(ident_sb[:]),
                    )
                aggT_sb = wpool.tile([P, F], s2_store, tag="aggTsb")
                nc.vector.tensor_copy(aggT_sb[:], aggT_ps[:])

                # stage 2: out[d, :128]  = agg_s @ WsrelT + x_s @ WsrootT (+bias)
                #          out[d, 128:]  = agg_v @ WvrelT + x_v @ WvrootT
                osv_ps = pmisc.tile([P, F], f32, tag="osv")
                nc.tensor.matmul(out=osv_ps[:, 0:H],
                                 lhsT=s2(aggT_sb[:, 0:P]), rhs=s2(wsrel_sb[:]),
                                 start=True, stop=False)
                nc.tensor.matmul(out=osv_ps[:, 0:H],
                                 lhsT=s2(xt_sb[:, c * P:(c + 1) * P]),
                                 rhs=s2(wsroot_sb[:]),
                                 start=False, stop=True)
                for kc in range(3):
                    nc.tensor.matmul(
                        out=osv_ps[:, H:F],
                        lhsT=s2(aggT_sb[:, (1 + kc) * P:(2 + kc) * P]),
                        rhs=s2(wvrel_sb[:, kc * 384:(kc + 1) * 384]),
                        start=(kc == 0), stop=False)
                for kc in range(3):
                    nc.tensor.matmul(
                        out=osv_ps[:, H:F],
                        lhsT=s2(xt_sb[:, (1 + kc) * NODES_PER_CORE + c * P:
                                      (1 + kc) * NODES_PER_CORE + (c + 1) * P]),
                        rhs=s2(wvroot_sb[:, kc * 384:(kc + 1) * 384]),
                        start=False, stop=(kc == 2))

                out_sb = wpool.tile([P, F], f32, tag="outsb")
                nc.vector.tensor_add(out_sb[:, 0:H], osv_ps[:, 0:H], bias_sb[:])
                nc.vector.tensor_copy(out_sb[:, H:F], osv_ps[:, H:F])
                nc.sync.dma_start(out[c * P:(c + 1) * P, :], out_sb[:])

    nc.finalize()
    return nc


def _get_program(T, cfg):
    key = (T, cfg)
    if key not in _prog_cache:
        _prog_cache[key] = _build_program(T, cfg)
    return _prog_cache[key]


def kernel(x, edge_index, W_scalar_rel, W_scalar_root, b_scalar_root,
           W_vector_rel, W_vector_root):
    cfg = tuple(CFG.split(","))
    s1_np = _npdt(cfg[0])
    s2_np = _npdt(cfg[1])

    x = np.asarray(x, dtype=np.float32)
    n = x.shape[0]
    assert n == N_NODES, x.shape
    row = np.asarray(edge_index[0], dtype=np.int64)
    col = np.asarray(edge_index[1], dtype=np.int64)

    # ---- host-side shard construction (sort edges by destination) ----
    order = np.argsort(row, kind="stable")
    row_s = row[order]
    col_s = col[order]
    bounds = np.searchsorted(row_s, np.arange(0, NP_PAD + 1, P))
    counts = np.diff(bounds)
    T = max(DEFAULT_T, int(np.ceil(counts.max() / P)))

    cap = T * P
    # padding edges point at the all-zero ZERO_ROW and rr=-1 (never matches
    # the iota, so their one-hot column is all-zero)
    cols_pad = np.full((N_CHUNKS, cap), ZERO_ROW, dtype=np.int16)
    rr_pad = np.full((N_CHUNKS, cap), -1.0, dtype=np.float32)
    for g in range(N_CHUNKS):
        s, e = bounds[g], bounds[g + 1]
        m = e - s
        if m:
            cols_pad[g, :m] = col_s[s:e]
            rr_pad[g, :m] = (row_s[s:e] - g * P).astype(np.float32)
    # dma_gather: flat edge i -> partition i % 128, tile-slot i // 128.
    # Each chunk's indices are split into NQ pieces of <= 1024; within a
    # piece, idx element j lives at wrapped position [j % 16, j // 16],
    # and the 16-row block is replicated across all 128 partitions
    # (the tx/rx Q7 cores each read their own 16-partition copy).
    GQ = 1024
    NQ = (cap + GQ - 1) // GQ
    WQ = GQ // 16
    cols_q = np.zeros((N_CHUNKS, NQ, GQ), dtype=np.int16)
    cols_q.reshape(N_CHUNKS, NQ * GQ)[:, :cap] = cols_pad
    wrap = cols_q.reshape(N_CHUNKS, NQ, WQ, 16).transpose(0, 1, 3, 2)  # [.., 16, WQ]
    cols_wrapped = np.tile(wrap, (1, 1, 8, 1))  # [N_CHUNKS, NQ, 128, WQ]
    cols_flat = cols_wrapped.transpose(0, 2, 1, 3).reshape(N_CHUNKS, P, NQ * WQ)
    cols_arr = np.ascontiguousarray(cols_flat).reshape(
        N_CORES, CHUNKS_PER_CORE, P, NQ * WQ)
    # rr for edge i goes to [i % 128, i // 128]
    rr_arr = rr_pad.reshape(N_CHUNKS, T, P).transpose(0, 2, 1)
    rr_arr = np.ascontiguousarray(rr_arr).reshape(N_CORES, CHUNKS_PER_CORE, P, T)

    x_flat = np.zeros((NP_PAD, F), dtype=np.float32)
    x_flat[:n] = x.reshape(n, F)
    xg_full = np.ascontiguousarray(x_flat.astype(s1_np))

    xT = x_flat.T  # [512, 10240], exact f32 for the root transform

    wsrelT = np.ascontiguousarray(np.asarray(W_scalar_rel, np.float32).T).astype(s2_np)
    wsrootT = np.ascontiguousarray(np.asarray(W_scalar_root, np.float32).T).astype(s2_np)
    wvrelT = np.ascontiguousarray(np.asarray(W_vector_rel, np.float32).T)
    wvrootT = np.ascontiguousarray(np.asarray(W_vector_root, np.float32).T)
    wvrel_packed = np.concatenate(
        [wvrelT[kc * P:(kc + 1) * P, :] for kc in range(3)], axis=1).astype(s2_np)
    wvroot_packed = np.concatenate(
        [wvrootT[kc * P:(kc + 1) * P, :] for kc in range(3)], axis=1).astype(s2_np)
    bias_t = np.ascontiguousarray(
        np.broadcast_to(np.asarray(b_scalar_root, np.float32), (P, H)))
    iota_t = np.ascontiguousarray(
        np.broadcast_to(np.arange(P, dtype=np.float32), (P, P)))
    ident_t = np.eye(P, dtype=np.float32).astype(s2_np)

    in_maps = []
    for core in range(N_CORES):
        base = core * NODES_PER_CORE
        xTc = xT[:, base:base + NODES_PER_CORE]  # [512, 1280]
        xTr = np.ascontiguousarray(
            xTc.reshape(4, P, NODES_PER_CORE).transpose(1, 0, 2)
               .reshape(P, 4 * NODES_PER_CORE)).astype(s2_np)
        in_maps.append({
            "xg": xg_full,
            "cols": np.ascontiguousarray(cols_arr[core]),
            "rr": np.ascontiguousarray(rr_arr[core]),
            "xt": xTr,
            "wsrel": wsrelT,
            "wsroot": wsrootT,
            "wvrel": wvrel_packed,
            "wvroot": wvroot_packed,
            "bias": bias_t,
            "iota": iota_t,
            "ident": ident_t,
        })

    nc = _get_program(T, cfg)
    kw = {}
    if PROFILE["on"]:
        kw = dict(trace=True, trace_cores=PROFILE["trace_cores"])
    res = run_bass_kernel_spmd(nc, in_maps, list(range(N_CORES)), **kw)
    PROFILE["last"] = res

    out_full = np.concatenate([res.results[i]["out"] for i in range(N_CORES)],
                              axis=0)
    return np.ascontiguousarray(
        out_full[:N_NODES].reshape(N_NODES, 4, H).astype(np.float32))


# revision 27
# speedup vs baseline: 2.0433x; 1.0346x over previous
"""Trainium2 Bass kernel for EquivariantGraphConv message passing.

Math (reference):
    scalar = x[:,0,:]; vector = x[:,1:,:].reshape(N, 3H)
    scalar_out = scalar @ Wsr.T + b + segsum(scalar[col] @ Wsrel.T, row)
    vector_out = vector @ Wvr.T + segsum(vector[col] @ Wvrel.T, row)

Key identity used: the edge transform is linear, so
    segsum(feat[col] @ W.T, row) == segsum(feat[col], row) @ W.T
We therefore aggregate the raw 512-dim node features per destination first
(16x fewer matmul FLOPs), then apply all four weight matrices per *node*.

Sharding: destinations are sharded across the 8 cores (1280 nodes each, in
10 chunks of 128).  Edges are sorted by destination on the host, so each
core only consumes its own edge shard and no cross-core reduction is
needed.  Each core gathers source features from a replicated padded node
table in DRAM with one big indirect DMA per chunk, builds one-hot
"selection" matrices on the vector engine (row_in_chunk == iota) and
matmul-accumulates P^T @ G into PSUM to realize the segment sum.
"""

import os
import sys

sys.path.insert(0, "/opt/trn_rl_repo")

import numpy as np
import ml_dtypes

import concourse.bass as bass
import concourse.mybir as mybir
import concourse.tile as tile
from concourse.bacc import Bacc
from concourse.bass_utils import run_bass_kernel_spmd

N_NODES = 10000
N_EDGES = 160000
H = 128
F = 4 * H            # 512 features per node (scalar 128 + vector 384)
P = 128              # partitions
NP_PAD = 10240       # padded node count (80 chunks of 128)
N_CORES = 8
NODES_PER_CORE = NP_PAD // N_CORES       # 1280
CHUNKS_PER_CORE = NODES_PER_CORE // P    # 10
N_CHUNKS = NP_PAD // P                   # 80
ZERO_ROW = N_NODES                       # padded zero row used by dummy edges
DEFAULT_T = 17                           # edge tiles per chunk (17*128 = 2176 cap)

# configuration: (gather/stage1 dtype, stage2 dtype); each of
# "bf16" | "f32" | "f32r".  f32r = fp32 storage, TF32-like matmul.
CFG = os.environ.get("BASS_GNN_CFG", "bf16,bf16")

# test.py hooks
PROFILE = {"on": False, "trace_cores": None, "last": None}

_prog_cache = {}


def _dt(name):
    return {
        "bf16": mybir.dt.bfloat16,
        "f32": mybir.dt.float32,
        "f32r": mybir.dt.float32,  # storage dtype; bitcast at matmul time
    }[name]


def _npdt(name):
    return {
        "bf16": ml_dtypes.bfloat16,
        "f32": np.float32,
        "f32r": np.float32,
    }[name]


def _build_program(T, cfg):
    """Build the (SPMD, per-core-identical) Bass program."""
    s1_name, s2_name = cfg
    s1_store = _dt(s1_name)
    s2_store = _dt(s2_name)

    def s1(ap):
        return ap.bitcast(mybir.dt.float32r) if s1_name == "f32r" else ap

    def s2(ap):
        return ap.bitcast(mybir.dt.float32r) if s2_name == "f32r" else ap

    nc = Bacc("TRN2", num_swdge_queues=4)
    f32 = mybir.dt.float32

    xg = nc.dram_tensor("xg", [NP_PAD, F], s1_store, kind="ExternalInput")
    # dma_gather is limited to ~1024 descriptors per instruction; split each
    # chunk's T*128 indices into NQ pieces of <= GQ indices.
    GQ = 1024
    NQ = (T * P + GQ - 1) // GQ
    WQ = GQ // 16  # idx columns per piece in the 16-partition wrapped layout
    cols = nc.dram_tensor("cols", [CHUNKS_PER_CORE, P, NQ * WQ], mybir.dt.int16,
                          kind="ExternalInput")
    rr = nc.dram_tensor("rr", [CHUNKS_PER_CORE, P, T], f32,
                        kind="ExternalInput")
    xt = nc.dram_tensor("xt", [P, 4 * NODES_PER_CORE], s2_store,
                        kind="ExternalInput")
    wsrel = nc.dram_tensor("wsrel", [P, H], s2_store, kind="ExternalInput")
    wsroot = nc.dram_tensor("wsroot", [P, H], s2_store, kind="ExternalInput")
    wvrel = nc.dram_tensor("wvrel", [P, 3 * 384], s2_store, kind="ExternalInput")
    wvroot = nc.dram_tensor("wvroot", [P, 3 * 384], s2_store, kind="ExternalInput")
    bias = nc.dram_tensor("bias", [P, H], f32, kind="ExternalInput")
    iota = nc.dram_tensor("iota", [P, P], f32, kind="ExternalInput")
    ident = nc.dram_tensor("ident", [P, P], s2_store, kind="ExternalInput")
    out = nc.dram_tensor("out", [NODES_PER_CORE, F], f32, kind="ExternalOutput")
    debug = os.environ.get("BASS_GNN_DEBUG", "0") == "1"
    if debug:
        dbg_agg = nc.dram_tensor("dbg_agg", [NODES_PER_CORE, F], f32,
                                 kind="ExternalOutput")
        dbg_g = nc.dram_tensor("dbg_g", [P, T * F], f32, kind="ExternalOutput")
        dbg_p = nc.dram_tensor("dbg_p", [P, T * P], f32, kind="ExternalOutput")

    with tile.TileContext(nc) as tc:
        with (
            tc.tile_pool(name="consts", bufs=1) as cpool,
            tc.tile_pool(name="edges", bufs=6) as epool,
            tc.tile_pool(name="gbuf", bufs=5) as gpool,
            tc.tile_pool(name="work", bufs=2) as wpool,
            tc.tile_pool(name="pagg", bufs=3, space="PSUM") as pagg,
            tc.tile_pool(name="pmisc", bufs=2, space="PSUM") as pmisc,
        ):
            # constants go on the Scalar HWDGE queue so the first chunk's
            # index DMAs (sync queue) aren't stuck behind the 2.6MB xt load
            xt_sb = cpool.tile([P, 4 * NODES_PER_CORE], s2_store)
            nc.scalar.dma_start(xt_sb[:], xt[:])
            wsrel_sb = cpool.tile([P, H], s2_store)
            nc.scalar.dma_start(wsrel_sb[:], wsrel[:])
            wsroot_sb = cpool.tile([P, H], s2_store)
            nc.scalar.dma_start(wsroot_sb[:], wsroot[:])
            wvrel_sb = cpool.tile([P, 3 * 384], s2_store)
            nc.scalar.dma_start(wvrel_sb[:], wvrel[:])
            wvroot_sb = cpool.tile([P, 3 * 384], s2_store)
            nc.scalar.dma_start(wvroot_sb[:], wvroot[:])
            bias_sb = cpool.tile([P, H], f32)
            nc.scalar.dma_start(bias_sb[:], bias[:])
            iota_sb = cpool.tile([P, P], f32)
            nc.scalar.dma_start(iota_sb[:], iota[:])
            ident_sb = cpool.tile([P, P], s2_store)
            nc.scalar.dma_start(ident_sb[:], ident[:])

            for c in range(CHUNKS_PER_CORE):
                cols_sb = epool.tile([P, NQ * WQ], mybir.dt.int16, tag="cols")
                nc.sync.dma_start(cols_sb[:], cols[c])
                rr_sb = epool.tile([P, T], f32, tag="rr")
                nc.sync.dma_start(rr_sb[:], rr[c])

                # gather: edge i -> G[i % 128, i // 128, :] = xg[cols_flat[i], :]
                G = gpool.tile([P, T * F], s1_store, tag="G")
                for q in range(NQ):
                    nidx = min(GQ, T * P - q * GQ)
                    nslots = nidx // P
                    nc.gpsimd.dma_gather(
                        G[:, q * (GQ // P) * F:
                             (q * (GQ // P) + nslots) * F]
                        .rearrange("p (t f) -> p t f", f=F),
                        xg[:],
                        cols_sb[:, q * WQ:(q + 1) * WQ],
                        nidx,
                        nidx,
                        F,
                        queue_num=(c * NQ + q) % 4,
                    )

                # one-hot P[p, t*128 + d] = (rr[p, t] == d)
                Pm = epool.tile([P, T * P], s1_store, tag="P")
                for t in range(T):
                    nc.vector.tensor_tensor(
                        out=Pm[:, t * P:(t + 1) * P],
                        in0=rr_sb[:, t:t + 1].to_broadcast([P, P]),
                        in1=iota_sb[:],
                        op=mybir.AluOpType.is_equal,
                    )

                # segment-sum: agg[d, f] = sum_t P_t^T @ G_t
                agg_ps = pagg.tile([P, F], f32, tag="agg")
                for t in range(T):
                    nc.tensor.matmul(
                        out=agg_ps[:],
                        lhsT=s1(Pm[:, t * P:(t + 1) * P]),
                        rhs=s1(G[:, t * F:(t + 1) * F]),
                        start=(t == 0),
                        stop=(t == T - 1),
                    )
                agg_sb = wpool.tile([P, F], s2_store, tag="aggsb")
                nc.vector.tensor_copy(agg_sb[:], agg_ps[:])
                if debug:
                    agg_f32_sb = wpool.tile([P, F], f32, tag="dbgagg")
                    nc.vector.tensor_copy(agg_f32_sb[:], agg_ps[:])
                    nc.sync.dma_start(dbg_agg[c * P:(c + 1) * P, :], agg_f32_sb[:])
                    if c == 0 and s1_store == f32:
                        nc.sync.dma_start(dbg_g[:], G[:])
                        nc.sync.dma_start(dbg_p[:], Pm[:])
                    elif c == 0:
                        g_f32_sb = wpool.tile([P, T * F], f32, tag="dbgg")
                        nc.vector.tensor_copy(g_f32_sb[:], G[:])
                        nc.sync.dma_start(dbg_g[:], g_f32_sb[:])
                        p_f32_sb = wpool.tile([P, T * P], f32, tag="dbgp")
                        nc.vector.tensor_copy(p_f32_sb[:], Pm[:])
                        nc.sync.dma_start(dbg_p[:], p_f32_sb[:])

                # transpose agg -> aggT[f, d] (4 PE transposes of 128x128)
                aggT_ps = pmisc.tile([P, F], s2_store, tag="aggT")
                for fc in range(4):
                    nc.tensor.transpose(
                        out=s2(aggT_ps[:, fc * P:(fc + 1) * P]),
                        in_=s2(agg_sb[:, fc * P:(fc + 1) * P]),
                        identity=s2(ident_sb[:]),
                    )
                aggT_sb = wpool.tile([P, F], s2_store, tag="aggTsb")
                nc.vector.tensor_copy(aggT_sb[:], aggT_ps[:])

                # stage 2: out[d, :128]  = agg_s @ WsrelT + x_s @ WsrootT (+bias)
                #          out[d, 128:]  = agg_v @ WvrelT + x_v @ WvrootT
                osv_ps = pmisc.tile([P, F], f32, tag="osv")
                nc.tensor.matmul(out=osv_ps[:, 0:H],
                                 lhsT=s2(aggT_sb[:, 0:P]), rhs=s2(wsrel_sb[:]),
                                 start=True, stop=False)
                nc.tensor.matmul(out=osv_ps[:, 0:H],
                                 lhsT=s2(xt_sb[:, c * P:(c + 1) * P]),
                                 rhs=s2(wsroot_sb[:]),
                                 start=False, stop=True)
                for kc in range(3):
                    nc.tensor.matmul(
                        out=osv_ps[:, H:F],
                        lhsT=s2(aggT_sb[:, (1 + kc) * P:(2 + kc) * P]),
                        rhs=s2(wvrel_sb[:, kc * 384:(kc + 1) * 384]),
                        start=(kc == 0), stop=False)
                for kc in range(3):
                    nc.tensor.matmul(
                        out=osv_ps[:, H:F],
                        lhsT=s2(xt_sb[:, (1 + kc) * NODES_PER_CORE + c * P:
                                      (1 + kc) * NODES_PER_CORE + (c + 1) * P]),
                        rhs=s2(wvroot_sb[:, kc * 384:(kc + 1) * 384]),
                        start=False, stop=(kc == 2))

                out_sb = wpool.tile([P, F], f32, tag="outsb")
                nc.vector.tensor_add(out_sb[:, 0:H], osv_ps[:, 0:H], bias_sb[:])
                nc.vector.tensor_copy(out_sb[:, H:F], osv_ps[:, H:F])
                nc.sync.dma_start(out[c * P:(c + 1) * P, :], out_sb[:])

    nc.finalize()
    return nc


def _get_program(T, cfg):
    key = (T, cfg)
    if key not in _prog_cache:
        _prog_cache[key] = _build_program(T, cfg)
    return _prog_cache[key]


def kernel(x, edge_index, W_scalar_rel, W_scalar_root, b_scalar_root,
           W_vector_rel, W_vector_root):
    cfg = tuple(CFG.split(","))
    s1_np = _npdt(cfg[0])
    s2_np = _npdt(cfg[1])

    x = np.asarray(x, dtype=np.float32)
    n = x.shape[0]
    assert n == N_NODES, x.shape
    row = np.asarray(edge_index[0], dtype=np.int64)
    col = np.asarray(edge_index[1], dtype=np.int64)

    # ---- host-side shard construction (sort edges by destination) ----
    order = np.argsort(row, kind="stable")
    row_s = row[order]
    col_s = col[order]
    bounds = np.searchsorted(row_s, np.arange(0, NP_PAD + 1, P))
    counts = np.diff(bounds)
    T = max(DEFAULT_T, int(np.ceil(counts.max() / P)))

    cap = T * P
    # padding edges point at the all-zero ZERO_ROW and rr=-1 (never matches
    # the iota, so their one-hot column is all-zero)
    cols_pad = np.full((N_CHUNKS, cap), ZERO_ROW, dtype=np.int16)
    rr_pad = np.full((N_CHUNKS, cap), -1.0, dtype=np.float32)
    for g in range(N_CHUNKS):
        s, e = bounds[g], bounds[g + 1]
        m = e - s
        if m:
            cols_pad[g, :m] = col_s[s:e]
            rr_pad[g, :m] = (row_s[s:e] - g * P).astype(np.float32)
    # dma_gather: flat edge i -> partition i % 128, tile-slot i // 128.
    # Each chunk's indices are split into NQ pieces of <= 1024; within a
    # piece, idx element j lives at wrapped position [j % 16, j // 16],
    # and the 16-row block is replicated across all 128 partitions
    # (the tx/rx Q7 cores each read their own 16-partition copy).
    GQ = 1024
    NQ = (cap + GQ - 1) // GQ
    WQ = GQ // 16
    cols_q = np.zeros((N_CHUNKS, NQ, GQ), dtype=np.int16)
    cols_q.reshape(N_CHUNKS, NQ * GQ)[:, :cap] = cols_pad
    wrap = cols_q.reshape(N_CHUNKS, NQ, WQ, 16).transpose(0, 1, 3, 2)  # [.., 16, WQ]
    cols_wrapped = np.tile(wrap, (1, 1, 8, 1))  # [N_CHUNKS, NQ, 128, WQ]
    cols_flat = cols_wrapped.transpose(0, 2, 1, 3).reshape(N_CHUNKS, P, NQ * WQ)
    cols_arr = np.ascontiguousarray(cols_flat).reshape(
        N_CORES, CHUNKS_PER_CORE, P, NQ * WQ)
    # rr for edge i goes to [i % 128, i // 128]
    rr_arr = rr_pad.reshape(N_CHUNKS, T, P).transpose(0, 2, 1)
    rr_arr = np.ascontiguousarray(rr_arr).reshape(N_CORES, CHUNKS_PER_CORE, P, T)

    x_flat = np.zeros((NP_PAD, F), dtype=np.float32)
    x_flat[:n] = x.reshape(n, F)
    xg_full = np.ascontiguousarray(x_flat.astype(s1_np))

    xT = x_flat.T  # [512, 10240], exact f32 for the root transform

    wsrelT = np.ascontiguousarray(np.asarray(W_scalar_rel, np.float32).T).astype(s2_np)
    wsrootT = np.ascontiguousarray(np.asarray(W_scalar_root, np.float32).T).astype(s2_np)
    wvrelT = np.ascontiguousarray(np.asarray(W_vector_rel, np.float32).T)
    wvrootT = np.ascontiguousarray(np.asarray(W_vector_root, np.float32).T)
    wvrel_packed = np.concatenate(
        [wvrelT[kc * P:(kc + 1) * P, :] for kc in range(3)], axis=1).astype(s2_np)
    wvroot_packed = np.concatenate(
        [wvrootT[kc * P:(kc + 1) * P, :] for kc in range(3)], axis=1).astype(s2_np)
    bias_t = np.ascontiguousarray(
        np.broadcast_to(np.asarray(b_scalar_root, np.float32), (P, H)))
    iota_t = np.ascontiguousarray(
        np.broadcast_to(np.arange(P, dtype=np.float32), (P, P)))
    ident_t = np.eye(P, dtype=np.float32).astype(s2_np)

    in_maps = []
    for core in range(N_CORES):
        base = core * NODES_PER_CORE
        xTc = xT[:, base:base + NODES_PER_CORE]  # [512, 1280]
        xTr = np.ascontiguousarray(
            xTc.reshape(4, P, NODES_PER_CORE).transpose(1, 0, 2)
               .reshape(P, 4 * NODES_PER_CORE)).astype(s2_np)
        in_maps.append({
            "xg": xg_full,
            "cols": np.ascontiguousarray(cols_arr[core]),
            "rr": np.ascontiguousarray(rr_arr[core]),
            "xt": xTr,
            "wsrel": wsrelT,
            "wsroot": wsrootT,
            "wvrel": wvrel_packed,
            "wvroot": wvroot_packed,
            "bias": bias_t,
            "iota": iota_t,
            "ident": ident_t,
        })

    nc = _get_program(T, cfg)
    kw = {}
    if PROFILE["on"]:
        kw = dict(trace=True, trace_cores=PROFILE["trace_cores"])
    res = run_bass_kernel_spmd(nc, in_maps, list(range(N_CORES)), **kw)
    PROFILE["last"] = res

    out_full = np.concatenate([res.results[i]["out"] for i in range(N_CORES)],
                              axis=0)
    return np.ascontiguousarray(
        out_full[:N_NODES].reshape(N_NODES, 4, H).astype(np.float32))


# revision 29
# speedup vs baseline: 2.1005x; 1.0280x over previous
"""Trainium2 Bass kernel for EquivariantGraphConv message passing.

Math (reference):
    scalar = x[:,0,:]; vector = x[:,1:,:].reshape(N, 3H)
    scalar_out = scalar @ Wsr.T + b + segsum(scalar[col] @ Wsrel.T, row)
    vector_out = vector @ Wvr.T + segsum(vector[col] @ Wvrel.T, row)

Key identity used: the edge transform is linear, so
    segsum(feat[col] @ W.T, row) == segsum(feat[col], row) @ W.T
We therefore aggregate the raw 512-dim node features per destination first
(16x fewer matmul FLOPs), then apply all four weight matrices per *node*.

Sharding: destinations are sharded across the 8 cores (1280 nodes each, in
10 chunks of 128).  Edges are sorted by destination on the host, so each
core only consumes its own edge shard and no cross-core reduction is
needed.  Each core gathers source features from a replicated padded node
table in DRAM with one big indirect DMA per chunk, builds one-hot
"selection" matrices on the vector engine (row_in_chunk == iota) and
matmul-accumulates P^T @ G into PSUM to realize the segment sum.
"""

import os
import sys

sys.path.insert(0, "/opt/trn_rl_repo")

import numpy as np
import ml_dtypes

import concourse.bass as bass
import concourse.mybir as mybir
import concourse.tile as tile
from concourse.bacc import Bacc
from concourse.bass_utils import run_bass_kernel_spmd

N_NODES = 10000
N_EDGES = 160000
H = 128
F = 4 * H            # 512 features per node (scalar 128 + vector 384)
P = 128              # partitions
NP_PAD = 10240       # padded node count (80 chunks of 128)
N_CORES = 8
NODES_PER_CORE = NP_PAD // N_CORES       # 1280
CHUNKS_PER_CORE = NODES_PER_CORE // P    # 10
N_CHUNKS = NP_PAD // P                   # 80
ZERO_ROW = N_NODES                       # padded zero row used by dummy edges
DEFAULT_T = 17                           # edge tiles per chunk (17*128 = 2176 cap)

# configuration: (gather/stage1 dtype, stage2 dtype); each of
# "bf16" | "f32" | "f32r".  f32r = fp32 storage, TF32-like matmul.
CFG = os.environ.get("BASS_GNN_CFG", "bf16,bf16")

# test.py hooks
PROFILE = {"on": False, "trace_cores": None, "last": None}

_prog_cache = {}


def _dt(name):
    return {
        "bf16": mybir.dt.bfloat16,
        "f32": mybir.dt.float32,
        "f32r": mybir.dt.float32,  # storage dtype; bitcast at matmul time
    }[name]


def _npdt(name):
    return {
        "bf16": ml_dtypes.bfloat16,
        "f32": np.float32,
        "f32r": np.float32,
    }[name]


def _build_program(T, cfg):
    """Build the (SPMD, per-core-identical) Bass program."""
    s1_name, s2_name = cfg
    s1_store = _dt(s1_name)
    s2_store = _dt(s2_name)

    def s1(ap):
        return ap.bitcast(mybir.dt.float32r) if s1_name == "f32r" else ap

    def s2(ap):
        return ap.bitcast(mybir.dt.float32r) if s2_name == "f32r" else ap

    nc = Bacc("TRN2", num_swdge_queues=4)
    f32 = mybir.dt.float32

    xg = nc.dram_tensor("xg", [NP_PAD, F], s1_store, kind="ExternalInput")
    # dma_gather is limited to ~1024 descriptors per instruction; split each
    # chunk's T*128 indices into NQ pieces of <= GQ indices.
    GQ = 1024
    NQ = (T * P + GQ - 1) // GQ
    WQ = GQ // 16  # idx columns per piece in the 16-partition wrapped layout
    cols = nc.dram_tensor("cols", [CHUNKS_PER_CORE, P, NQ * WQ], mybir.dt.int16,
                          kind="ExternalInput")
    rr = nc.dram_tensor("rr", [CHUNKS_PER_CORE, P, T], f32,
                        kind="ExternalInput")
    xt = nc.dram_tensor("xt", [P, 4 * NODES_PER_CORE], s2_store,
                        kind="ExternalInput")
    wsrel = nc.dram_tensor("wsrel", [P, H], s2_store, kind="ExternalInput")
    wsroot = nc.dram_tensor("wsroot", [P, H], s2_store, kind="ExternalInput")
    wvrel = nc.dram_tensor("wvrel", [P, 3 * 384], s2_store, kind="ExternalInput")
    wvroot = nc.dram_tensor("wvroot", [P, 3 * 384], s2_store, kind="ExternalInput")
    bias = nc.dram_tensor("bias", [P, H], f32, kind="ExternalInput")
    iota = nc.dram_tensor("iota", [P, P], f32, kind="ExternalInput")
    ident = nc.dram_tensor("ident", [P, P], s2_store, kind="ExternalInput")
    out = nc.dram_tensor("out", [NODES_PER_CORE, F], f32, kind="ExternalOutput")
    debug = os.environ.get("BASS_GNN_DEBUG", "0") == "1"
    if debug:
        dbg_agg = nc.dram_tensor("dbg_agg", [NODES_PER_CORE, F], f32,
                                 kind="ExternalOutput")
        dbg_g = nc.dram_tensor("dbg_g", [P, T * F], f32, kind="ExternalOutput")
        dbg_p = nc.dram_tensor("dbg_p", [P, T * P], f32, kind="ExternalOutput")

    with tile.TileContext(nc) as tc:
        with (
            tc.tile_pool(name="consts", bufs=1) as cpool,
            tc.tile_pool(name="edges", bufs=6) as epool,
            tc.tile_pool(name="gbuf", bufs=5) as gpool,
            tc.tile_pool(name="work", bufs=4) as wpool,
            tc.tile_pool(name="pagg", bufs=3, space="PSUM") as pagg,
            tc.tile_pool(name="pmisc", bufs=2, space="PSUM") as pmisc,
        ):
            # constants go on the Scalar HWDGE queue so the first chunk's
            # index DMAs (sync queue) aren't stuck behind the 2.6MB xt load
            xt_sb = cpool.tile([P, 4 * NODES_PER_CORE], s2_store)
            nc.scalar.dma_start(xt_sb[:], xt[:])
            wsrel_sb = cpool.tile([P, H], s2_store)
            nc.scalar.dma_start(wsrel_sb[:], wsrel[:])
            wsroot_sb = cpool.tile([P, H], s2_store)
            nc.scalar.dma_start(wsroot_sb[:], wsroot[:])
            wvrel_sb = cpool.tile([P, 3 * 384], s2_store)
            nc.scalar.dma_start(wvrel_sb[:], wvrel[:])
            wvroot_sb = cpool.tile([P, 3 * 384], s2_store)
            nc.scalar.dma_start(wvroot_sb[:], wvroot[:])
            bias_sb = cpool.tile([P, H], f32)
            nc.scalar.dma_start(bias_sb[:], bias[:])
            iota_sb = cpool.tile([P, P], f32)
            nc.scalar.dma_start(iota_sb[:], iota[:])
            ident_sb = cpool.tile([P, P], s2_store)
            nc.scalar.dma_start(ident_sb[:], ident[:])

            LAG = 2  # stage-2 for chunk c-LAG runs amid stage-1 of chunk c
            agg_tiles = {}

            def stage1(c):
                cols_sb = epool.tile([P, NQ * WQ], mybir.dt.int16, tag="cols")
                nc.sync.dma_start(cols_sb[:], cols[c])
                rr_sb = epool.tile([P, T], f32, tag="rr")
                nc.sync.dma_start(rr_sb[:], rr[c])

                # gather: edge i -> G[i % 128, i // 128, :] = xg[cols_flat[i], :]
                G = gpool.tile([P, T * F], s1_store, tag="G")
                for q in range(NQ):
                    nidx = min(GQ, T * P - q * GQ)
                    nslots = nidx // P
                    nc.gpsimd.dma_gather(
                        G[:, q * (GQ // P) * F:
                             (q * (GQ // P) + nslots) * F]
                        .rearrange("p (t f) -> p t f", f=F),
                        xg[:],
                        cols_sb[:, q * WQ:(q + 1) * WQ],
                        nidx,
                        nidx,
                        F,
                        queue_num=(c * NQ + q) % 4,
                    )

                # one-hot P[p, t*128 + d] = (rr[p, t] == d)
                Pm = epool.tile([P, T * P], s1_store, tag="P")
                for t in range(T):
                    nc.vector.tensor_tensor(
                        out=Pm[:, t * P:(t + 1) * P],
                        in0=rr_sb[:, t:t + 1].to_broadcast([P, P]),
                        in1=iota_sb[:],
                        op=mybir.AluOpType.is_equal,
                    )

                # segment-sum: agg[d, f] = sum_t P_t^T @ G_t
                agg_ps = pagg.tile([P, F], f32, tag="agg")
                for t in range(T):
                    nc.tensor.matmul(
                        out=agg_ps[:],
                        lhsT=s1(Pm[:, t * P:(t + 1) * P]),
                        rhs=s1(G[:, t * F:(t + 1) * F]),
                        start=(t == 0),
                        stop=(t == T - 1),
                    )
                agg_sb = wpool.tile([P, F], s2_store, tag="aggsb")
                nc.vector.tensor_copy(agg_sb[:], agg_ps[:])
                agg_tiles[c] = agg_sb
                if debug:
                    agg_f32_sb = wpool.tile([P, F], f32, tag="dbgagg")
                    nc.vector.tensor_copy(agg_f32_sb[:], agg_ps[:])
                    nc.sync.dma_start(dbg_agg[c * P:(c + 1) * P, :], agg_f32_sb[:])

            def stage2(c):
                agg_sb = agg_tiles.pop(c)
                # transpose agg -> aggT[f, d] (4 PE transposes of 128x128)
                aggT_ps = pmisc.tile([P, F], s2_store, tag="aggT")
                for fc in range(4):
                    nc.tensor.transpose(
                        out=s2(aggT_ps[:, fc * P:(fc + 1) * P]),
                        in_=s2(agg_sb[:, fc * P:(fc + 1) * P]),
                        identity=s2(ident_sb[:]),
                    )
                aggT_sb = wpool.tile([P, F], s2_store, tag="aggTsb")
                nc.vector.tensor_copy(aggT_sb[:], aggT_ps[:])

                # stage 2: out[d, :128]  = agg_s @ WsrelT + x_s @ WsrootT (+bias)
                #          out[d, 128:]  = agg_v @ WvrelT + x_v @ WvrootT
                osv_ps = pmisc.tile([P, F], f32, tag="osv")
                nc.tensor.matmul(out=osv_ps[:, 0:H],
                                 lhsT=s2(aggT_sb[:, 0:P]), rhs=s2(wsrel_sb[:]),
                                 start=True, stop=False)
                nc.tensor.matmul(out=osv_ps[:, 0:H],
                                 lhsT=s2(xt_sb[:, c * P:(c + 1) * P]),
                                 rhs=s2(wsroot_sb[:]),
                                 start=False, stop=True)
                for kc in range(3):
                    nc.tensor.matmul(
                        out=osv_ps[:, H:F],
                        lhsT=s2(aggT_sb[:, (1 + kc) * P:(2 + kc) * P]),
                        rhs=s2(wvrel_sb[:, kc * 384:(kc + 1) * 384]),
                        start=(kc == 0), stop=False)
                for kc in range(3):
                    nc.tensor.matmul(
                        out=osv_ps[:, H:F],
                        lhsT=s2(xt_sb[:, (1 + kc) * NODES_PER_CORE + c * P:
                                      (1 + kc) * NODES_PER_CORE + (c + 1) * P]),
                        rhs=s2(wvroot_sb[:, kc * 384:(kc + 1) * 384]),
                        start=False, stop=(kc == 2))

                out_sb = wpool.tile([P, F], f32, tag="outsb")
                nc.vector.tensor_add(out_sb[:, 0:H], osv_ps[:, 0:H], bias_sb[:])
                nc.vector.tensor_copy(out_sb[:, H:F], osv_ps[:, H:F])
                nc.sync.dma_start(out[c * P:(c + 1) * P, :], out_sb[:])

            for c in range(CHUNKS_PER_CORE + LAG):
                if c < CHUNKS_PER_CORE:
                    stage1(c)
                if c >= LAG:
                    stage2(c - LAG)

    nc.finalize()
    return nc


def _get_program(T, cfg):
    key = (T, cfg)
    if key not in _prog_cache:
        _prog_cache[key] = _build_program(T, cfg)
    return _prog_cache[key]


def kernel(x, edge_index, W_scalar_rel, W_scalar_root, b_scalar_root,
           W_vector_rel, W_vector_root):
    cfg = tuple(CFG.split(","))
    s1_np = _npdt(cfg[0])
    s2_np = _npdt(cfg[1])

    x = np.asarray(x, dtype=np.float32)
    n = x.shape[0]
    assert n == N_NODES, x.shape
    row = np.asarray(edge_index[0], dtype=np.int64)
    col = np.asarray(edge_index[1], dtype=np.int64)

    # ---- host-side shard construction (sort edges by destination) ----
    order = np.argsort(row, kind="stable")
    row_s = row[order]
    col_s = col[order]
    bounds = np.searchsorted(row_s, np.arange(0, NP_PAD + 1, P))
    counts = np.diff(bounds)
    T = max(DEFAULT_T, int(np.ceil(counts.max() / P)))

    cap = T * P
    # padding edges point at the all-zero ZERO_ROW and rr=-1 (never matches
    # the iota, so their one-hot column is all-zero)
    cols_pad = np.full((N_CHUNKS, cap), ZERO_ROW, dtype=np.int16)
    rr_pad = np.full((N_CHUNKS, cap), -1.0, dtype=np.float32)
    for g in range(N_CHUNKS):
        s, e = bounds[g], bounds[g + 1]
        m = e - s
        if m:
            cols_pad[g, :m] = col_s[s:e]
            rr_pad[g, :m] = (row_s[s:e] - g * P).astype(np.float32)
    # dma_gather: flat edge i -> partition i % 128, tile-slot i // 128.
    # Each chunk's indices are split into NQ pieces of <= 1024; within a
    # piece, idx element j lives at wrapped position [j % 16, j // 16],
    # and the 16-row block is replicated across all 128 partitions
    # (the tx/rx Q7 cores each read their own 16-partition copy).
    GQ = 1024
    NQ = (cap + GQ - 1) // GQ
    WQ = GQ // 16
    cols_q = np.zeros((N_CHUNKS, NQ, GQ), dtype=np.int16)
    cols_q.reshape(N_CHUNKS, NQ * GQ)[:, :cap] = cols_pad
    wrap = cols_q.reshape(N_CHUNKS, NQ, WQ, 16).transpose(0, 1, 3, 2)  # [.., 16, WQ]
    cols_wrapped = np.tile(wrap, (1, 1, 8, 1))  # [N_CHUNKS, NQ, 128, WQ]
    cols_flat = cols_wrapped.transpose(0, 2, 1, 3).reshape(N_CHUNKS, P, NQ * WQ)
    cols_arr = np.ascontiguousarray(cols_flat).reshape(
        N_CORES, CHUNKS_PER_CORE, P, NQ * WQ)
    # rr for edge i goes to [i % 128, i // 128]
    rr_arr = rr_pad.reshape(N_CHUNKS, T, P).transpose(0, 2, 1)
    rr_arr = np.ascontiguousarray(rr_arr).reshape(N_CORES, CHUNKS_PER_CORE, P, T)

    x_flat = np.zeros((NP_PAD, F), dtype=np.float32)
    x_flat[:n] = x.reshape(n, F)
    xg_full = np.ascontiguousarray(x_flat.astype(s1_np))

    xT = x_flat.T  # [512, 10240], exact f32 for the root transform

    wsrelT = np.ascontiguousarray(np.asarray(W_scalar_rel, np.float32).T).astype(s2_np)
    wsrootT = np.ascontiguousarray(np.asarray(W_scalar_root, np.float32).T).astype(s2_np)
    wvrelT = np.ascontiguousarray(np.asarray(W_vector_rel, np.float32).T)
    wvrootT = np.ascontiguousarray(np.asarray(W_vector_root, np.float32).T)
    wvrel_packed = np.concatenate(
        [wvrelT[kc * P:(kc + 1) * P, :] for kc in range(3)], axis=1).astype(s2_np)
    wvroot_packed = np.concatenate(
        [wvrootT[kc * P:(kc + 1) * P, :] for kc in range(3)], axis=1).astype(s2_np)
    bias_t = np.ascontiguousarray(
        np.broadcast_to(np.asarray(b_scalar_root, np.float32), (P, H)))
    iota_t = np.ascontiguousarray(
        np.broadcast_to(np.arange(P, dtype=np.float32), (P, P)))
    ident_t = np.eye(P, dtype=np.float32).astype(s2_np)

    in_maps = []
    for core in range(N_CORES):
        base = core * NODES_PER_CORE
        xTc = xT[:, base:base + NODES_PER_CORE]  # [512, 1280]
        xTr = np.ascontiguousarray(
            xTc.reshape(4, P, NODES_PER_CORE).transpose(1, 0, 2)
               .reshape(P, 4 * NODES_PER_CORE)).astype(s2_np)
        in_maps.append({
            "xg": xg_full,
            "cols": np.ascontiguousarray(cols_arr[core]),
            "rr": np.ascontiguousarray(rr_arr[core]),
            "xt": xTr,
            "wsrel": wsrelT,
            "wsroot": wsrootT,
            "wvrel": wvrel_packed,
            "wvroot": wvroot_packed,
            "bias": bias_t,
            "iota": iota_t,
            "ident": ident_t,
        })

    nc = _get_program(T, cfg)
    kw = {}
    if PROFILE["on"]:
        kw = dict(trace=True, trace_cores=PROFILE["trace_cores"])
    res = run_bass_kernel_spmd(nc, in_maps, list(range(N_CORES)), **kw)
    PROFILE["last"] = res

    out_full = np.concatenate([res.results[i]["out"] for i in range(N_CORES)],
                              axis=0)
    return np.ascontiguousarray(
        out_full[:N_NODES].reshape(N_NODES, 4, H).astype(np.float32))
